# Initial kernel scaffold
#
"""Trainium2 Bass kernel for nn_GAT_GCN (GAT conv + GCN conv + pooling + MLP tail).

Strategy (8 NeuronCores, SPMD, full inputs in / full output out):
  - Nodes are sharded by graph: core c owns graphs [16c, 16c+16). Within a core,
    nodes are laid out in graph-aligned "slots" (MAXG slots per graph) so pooling
    reduces over fixed-size slot ranges (uniform program across cores).
  - Edges are sharded by destination node. Per 128-slot destination block, edges
    are processed in subtiles of 128; a host-built one-hot mask turns the
    per-destination segment-sum into a TensorEngine matmul.
  - GAT: gather per-edge payload [x[src] | a_src[src]] (256 B rows, bf16) from a
    replicated table; attention softmax folded into the mask matmul (exp(e)
    weights in the rhs, normalization by the per-destination sum afterwards).
    The head-blocked weight multiply (y @ W_h) runs as a separate pass using
    DMA-transpose loads.
  - GCN: z rows (scaled by dinv[src] at production time) are gathered directly
    as the matmul rhs; dinv[dst] is applied when copying out of PSUM. The dense
    [1140x1140] multiply runs as a transposed pass producing zfin^T, which
    feeds max/mean pooling via free-dim reductions.
  - Cross-core exchange: AllGather of the small payload table and of z
    (bf16), interleaved across the two branches to hide latency.
  - The tiny MLP tail is computed replicated on every core.

Host-side preprocessing is restricted to index manipulation (sorting/bucketing
edges, one-hot masks, padding) and parameter repacking (padding / bf16 casts /
tiny reshapes of weights) -- all data-dependent float compute runs on device.
"""

import math

import numpy as np
import ml_dtypes

import concourse.bacc as bacc
import concourse.bass as bass
import concourse.tile as tile
from concourse import mybir, library_config
from concourse.bass_utils import run_bass_kernel_spmd
from concourse.tile_rust import add_dep_helper

# ---------------------------------------------------------------- constants
N = 20000
E = 160000
G = 128
F = 114
H = 10
HF = 1140          # F * H
NC = 8
GPC = G // NC      # graphs per core
P = 128

bf16 = mybir.dt.bfloat16
f32 = mybir.dt.float32
i16 = mybir.dt.int16

BF = ml_dtypes.bfloat16

_PROG_CACHE: dict = {}


# ---------------------------------------------------------------- host utils
def _wrap_idx(idx: np.ndarray) -> np.ndarray:
    """int16 index list (len % 16 == 0) -> [128, len/16] wrapped layout."""
    n = idx.shape[0]
    assert n % 16 == 0
    return np.tile(idx.reshape(-1, 16).T, (8, 1)).astype(np.int16)


def _prep_branch(x, ei, batch, gatW, att_src, att_dst):
    """Host preprocessing for one branch. Returns (static, percore, shared)."""
    x = np.asarray(x, dtype=np.float32)
    ei = np.asarray(ei).astype(np.int64)
    batch = np.asarray(batch).astype(np.int64)
    gatW = np.asarray(gatW, dtype=np.float32)
    att_src = np.asarray(att_src, dtype=np.float32)
    att_dst = np.asarray(att_dst, dtype=np.float32)

    cnt = np.bincount(batch, minlength=G)
    MAXG = int(cnt.max())
    SS = ((16 * MAXG + 127) // 128) * 128       # slots per core shard
    NBLK = SS // 128
    assert NC * SS < 32768, "row ids must fit int16"

    gstart = np.zeros(G + 1, np.int64)
    np.cumsum(cnt, out=gstart[1:])
    nodes = np.arange(N)
    rank = nodes - gstart[batch]
    slot_of_node = (batch % GPC) * MAXG + rank          # [N] in [0, 16*MAXG)
    core_of_node = batch // GPC                          # [N]
    row_of_node = core_of_node * SS + slot_of_node       # [N] global table row

    src = np.concatenate([ei[0], nodes])
    dst = np.concatenate([ei[1], nodes])
    NE = src.shape[0]

    core_e = core_of_node[dst]
    slot_e = slot_of_node[dst]
    blk_e = slot_e // 128
    dloc_e = slot_e % 128

    # bucket edges per (core, block)
    order = np.lexsort((blk_e, core_e))
    src_s, core_s, blk_s, dloc_s = src[order], core_e[order], blk_e[order], dloc_e[order]
    key = core_s * NBLK + blk_s
    counts = np.bincount(key, minlength=NC * NBLK)
    starts = np.zeros(NC * NBLK + 1, np.int64)
    np.cumsum(counts, out=starts[1:])
    NSUB = int(max(1, -(-counts.max() // 128)))
    CAP = NSUB * 128

    percore = []
    for c in range(NC):
        isrc = np.zeros((NBLK, CAP), np.int64)
        mask = np.zeros((NBLK, 128, NSUB, 128), np.float32)   # [e, s, dloc]
        for b in range(NBLK):
            k = c * NBLK + b
            cnt_b = counts[k]
            sl = slice(starts[k], starts[k + 1])
            isrc[b, :cnt_b] = row_of_node[src_s[sl]]
            pos = np.arange(cnt_b)
            mask[b, pos % 128, pos // 128, dloc_s[sl]] = 1.0
        maskT = np.ascontiguousarray(mask.transpose(0, 3, 2, 1))  # [d, s, e]
        isrc_w = np.stack([_wrap_idx(isrc[b].astype(np.int16)) for b in range(NBLK)])

        # per-slot metadata for this core
        slots = np.arange(SS)
        g_local = np.minimum(slots // MAXG, GPC - 1)
        r_local = slots - g_local * MAXG
        g_global = c * GPC + g_local
        validity = (slots < 16 * MAXG) & (r_local < cnt[g_global])
        node_of_slot = np.zeros(SS, np.int64)
        real = validity.nonzero()[0]
        node_of_slot[real] = gstart[g_global[real]] + r_local[real]
        xidx = np.stack([_wrap_idx(node_of_slot[b * 128:(b + 1) * 128].astype(np.int16))
                         for b in range(NBLK)])

        deg = np.bincount(dst, minlength=N).astype(np.float64)
        dinv_node = 1.0 / np.sqrt(np.maximum(deg, 1.0))
        dinv_slot = np.ones(SS, np.float32)
        dinv_slot[real] = dinv_node[node_of_slot[real]]

        valid = validity.astype(np.float32)
        poison = np.where(validity, 0.0, -1e28).astype(np.float32)
        rcnt = (1.0 / np.maximum(cnt[c * GPC:(c + 1) * GPC], 1.0)).astype(np.float32)

        percore.append({
            "isrc": isrc_w.astype(np.int16),                        # [NBLK,128,NSUB*8]
            "mask": mask.astype(BF),                                # [NBLK,128,NSUB,128]
            "maskT": maskT.astype(BF),
            "xidx": xidx.astype(np.int16),                          # [NBLK,128,8]
            "dinv": dinv_slot.reshape(NBLK, 128, 1),                # f32
            "valid": valid.reshape(NBLK, 128, 1),
            "inval": (1.0 - valid).reshape(NBLK, 128, 1),
            "poison": np.broadcast_to(poison.astype(BF), (128, SS)).copy(),
            "rcnt": np.broadcast_to(rcnt, (128, GPC)).copy(),
        })

    # shared (weights / x table)
    xpad = np.zeros((N, 128), np.float32)
    xpad[:, :F] = x
    gatW3 = gatW.reshape(F, H, F)
    Ws = np.einsum("khf,hf->kh", gatW3, att_src)
    Wd = np.einsum("khf,hf->kh", gatW3, att_dst)
    Wsd = np.zeros((128, 2 * H), np.float32)
    Wsd[:F, :H] = Ws
    Wsd[:F, H:] = Wd

    shared = {
        "xpad": xpad.astype(BF),              # [N,128] bf16
        "Wsd": Wsd.astype(BF),                # [128,20]
    }
    static = dict(MAXG=MAXG, SS=SS, NBLK=NBLK, NSUB=NSUB)
    return static, percore, shared


def _pack_branch_weights(gatW, gatb, gcnW, gcnb):
    gatW = np.asarray(gatW, np.float32)
    gatb = np.asarray(gatb, np.float32)
    gcnW = np.asarray(gcnW, np.float32)
    gcnb = np.asarray(gcnb, np.float32)
    gatWk = np.zeros((H, 128, F), np.float32)
    gatW3 = gatW.reshape(F, H, F)
    for h in range(H):
        gatWk[h, :F, :] = gatW3[:, h, :]
        gatWk[h, F, :] = gatb[h * F:(h + 1) * F]
    gcn_pad = np.zeros((1152, 1152), np.float32)
    gcn_pad[:HF, :HF] = gcnW
    gcn_pad[HF, :HF] = gcnb
    gcnWk = gcn_pad.reshape(9, 128, 1152)
    return gatWk.astype(BF), gcnWk.astype(BF)


def _pack_tail(inp):
    f = lambda k: np.asarray(inp[k], np.float32)
    tail = {}
    for p in ("p1", "p2"):
        W1 = np.zeros((2432, 1024), np.float32)
        fg1 = f(p + "_fcg1W")           # [2280, 1000]
        W1[0:HF, 0:1000] = fg1[0:HF]
        W1[1152:1152 + HF, 0:1000] = fg1[HF:2 * HF]
        k1 = np.concatenate([W1[:2304].reshape(18, 128, 1024),
                             np.zeros((1, 128, 1024), np.float32)], axis=0)
        k1[18, 0, 0:1000] = f(p + "_fcg1b")
        tail[p + "_fcg1Wk"] = k1.astype(np.float32)
        W2 = np.zeros((1024, 64), np.float32)
        W2[0:1000] = f(p + "_fcg2W")
        W2[1000] = f(p + "_fcg2b")
        tail[p + "_fcg2Wk"] = W2.reshape(8, 128, 64).astype(np.float32)
    Wx = np.zeros((1024, 128), np.float32)
    Wx[0:1000] = f("fcxtW")
    Wx[1000] = f("fcxtb")
    tail["fcxtWk"] = Wx.reshape(8, 128, 128).astype(np.float32)
    W1 = np.zeros((3, 128, 128), np.float32)
    W1[0] = f("fc1W")[0:128]
    W1[1] = f("fc1W")[128:256]
    W1[2, 0] = f("fc1b")
    tail["fc1Wk"] = W1.astype(np.float32)
    W2 = np.zeros((2, 128, 32), np.float32)
    W2[0] = f("fc2W")
    W2[1, 0] = f("fc2b")
    tail["fc2Wk"] = W2.astype(np.float32)
    Wo = np.zeros((128, 1), np.float32)
    Wo[0:32, 0] = f("outW")[:, 0]
    Wo[32, 0] = float(np.asarray(inp["outb"]).reshape(-1)[0])
    tail["outWk"] = Wo.astype(np.float32)
    tail["identity"] = np.eye(128, dtype=np.float32)
    tail["target"] = f("target")
    return tail


# ---------------------------------------------------------------- device build
GATHER_MAX = 1024  # dma_gather breaks above 1024 indices per call (HW-probed)


def _gather_chunked(nc, pools, out_tile, table_ap, ii, cap, elem, deps=()):
    """Emit dma_gather in <=1024-index chunks. out_tile is [128, cap/128, elem]."""
    insts = []
    for i0 in range(0, cap, GATHER_MAX):
        i1 = min(cap, i0 + GATHER_MAX)
        n = i1 - i0
        g = nc.gpsimd.dma_gather(
            out_tile[:, i0 // 128:i1 // 128, :], table_ap,
            ii[:, i0 // 16:i1 // 16], n, n, elem)
        add_dep_helper(g.ins, pools["lib"].ins, reason="gather after lib")
        for dd in deps:
            add_dep_helper(g.ins, dd.ins, reason="gather dep")
        insts.append(g)
    return insts


def _build_branch(nc, tc, ctx, pools, pfx, st, deps):
    """Emit phases T-build, AG_T, GAT-agg, GAT-W, AG_z for one branch.

    Returns a dict with handles needed by the GCN phases.
    """
    MAXG, SS, NBLK, NSUB = st["MAXG"], st["SS"], st["NBLK"], st["NSUB"]
    CAP = NSUB * 128
    d = nc.dram_tensor
    # inputs
    xpad = d(pfx + "xpad", [N, 128], bf16, kind="ExternalInput")
    xidx = d(pfx + "xidx", [NBLK, 128, 8], i16, kind="ExternalInput")
    isrc = d(pfx + "isrc", [NBLK, 128, NSUB * 8], i16, kind="ExternalInput")
    maskD = d(pfx + "mask", [NBLK, 128, NSUB, 128], bf16, kind="ExternalInput")
    maskTD = d(pfx + "maskT", [NBLK, 128, NSUB, 128], bf16, kind="ExternalInput")
    dinvD = d(pfx + "dinv", [NBLK, 128, 1], f32, kind="ExternalInput")
    invalD = d(pfx + "inval", [NBLK, 128, 1], f32, kind="ExternalInput")
    validD = d(pfx + "valid", [NBLK, 128, 1], f32, kind="ExternalInput")
    WsdD = d(pfx + "Wsd", [128, 2 * H], bf16, kind="ExternalInput")
    gatWkD = d(pfx + "gatWk", [H, 128, F], bf16, kind="ExternalInput")
    # internals
    T_loc = d(pfx + "T_loc", [SS, 256], bf16)
    T_glob = d(pfx + "T_glob", [NC * SS, 256], bf16, addr_space="Shared")
    y_dram = d(pfx + "y", [SS, 1290], bf16)
    z_loc = d(pfx + "z_loc", [SS, 1152], bf16)
    z_glob = d(pfx + "z_glob", [NC * SS, 1152], bf16, addr_space="Shared")

    sb, ps = pools["sb"], pools["ps"]
    rg = [list(range(NC))]

    # weights resident for this branch section
    Wsd = sb.tile([128, 2 * H], bf16, tag="wsd")
    nc.sync.dma_start(Wsd[:], WsdD[:])
    gatWk = sb.tile([128, H, F], bf16, tag="gatwk")
    nc.sync.dma_start(gatWk[:], gatWkD.ap().rearrange("h k n -> k h n"))

    # ---------------- Phase 1: T-build ----------------
    t_writes = []
    for b in range(NBLK):
        xi = sb.tile([128, 8], i16, tag="xi")
        nc.sync.dma_start(xi[:], xidx[b])
        xg = sb.tile([128, 1, 128], bf16, tag="xg")
        g1 = nc.gpsimd.dma_gather(xg[:], xpad[:], xi[:], 128, 128, 128)
        add_dep_helper(g1.ins, pools["lib"].ins, reason="gather after lib")
        xgT = sb.tile([128, 1, 128], bf16, tag="xgT")
        g2 = nc.gpsimd.dma_gather(xgT[:], xpad[:], xi[:], 128, 128, 128, transpose=True)
        add_dep_helper(g2.ins, pools["lib"].ins, reason="gather after lib")
        aps = ps.tile([128, 2 * H], f32, tag="ps_small")
        nc.tensor.matmul(aps[:], xgT[:, 0, :], Wsd[:], start=True, stop=True)
        Tt = sb.tile([128, 256], bf16, tag="Tt")
        nc.vector.tensor_copy(Tt[:, 0:F], xg[:, 0, 0:F])
        nc.vector.tensor_copy(Tt[:, F:F + 2 * H], aps[:])
        nc.vector.memset(Tt[:, F + 2 * H:256], 0.0)
        w = nc.sync.dma_start(T_loc[b * 128:(b + 1) * 128, :], Tt[:])
        t_writes.append(w)
    ag_t = nc.gpsimd.collective_compute(
        "AllGather", mybir.AluOpType.bypass, replica_groups=rg,
        ins=[T_loc[:]], outs=[T_glob[:]])
    for w in t_writes:
        add_dep_helper(ag_t.ins, w.ins, reason="AG_T after T writes")

    # ---------------- Phase 2: GAT aggregation ----------------
    y_writes = []
    for b in range(NBLK):
        ii = sb.tile([128, NSUB * 8], i16, tag="ii")
        nc.sync.dma_start(ii[:], isrc[b])
        S = sb.tile([128, NSUB, 256], bf16, tag="S")
        _gather_chunked(nc, pools, S, T_glob[:], ii, CAP, 256, deps=(ag_t,))
        Mt = sb.tile([128, NSUB, 128], bf16, tag="Mt")
        nc.sync.dma_start(Mt[:], maskD[b])
        MtT = sb.tile([128, NSUB, 128], bf16, tag="MtT")
        nc.sync.dma_start(MtT[:], maskTD[b])
        adb = sb.tile([128, 2 * H], bf16, tag="adb")
        r = nc.sync.dma_start(adb[:], T_loc[b * 128:(b + 1) * 128, F:F + 2 * H])
        add_dep_helper(r.ins, t_writes[b].ins, reason="adb after T write")
        inval = sb.tile([128, 1], f32, tag="col")
        nc.sync.dma_start(inval[:], invalD[b])
        dinv = sb.tile([128, 1], f32, tag="col2")
        nc.sync.dma_start(dinv[:], dinvD[b])

        lg = sb.tile([128, NSUB, H], f32, tag="lg")
        for s in range(NSUB):
            ad_ps = ps.tile([128, H], f32, tag="ps_small")
            nc.tensor.matmul(ad_ps[:], MtT[:, s, :], adb[:, H:2 * H],
                             start=True, stop=True)
            nc.vector.tensor_tensor(out=lg[:, s, :], in0=S[:, s, F:F + H],
                                    in1=ad_ps[:], op=mybir.AluOpType.add)
        l3 = sb.tile([128, NSUB, H], f32, tag="l3")
        nc.vector.scalar_tensor_tensor(out=l3[:], in0=lg[:], scalar=0.2, in1=lg[:],
                                       op0=mybir.AluOpType.mult,
                                       op1=mybir.AluOpType.max)
        exb = sb.tile([128, NSUB, H], bf16, tag="exb")
        nc.scalar.activation(exb[:], l3[:], mybir.ActivationFunctionType.Exp)

        R = sb.tile([128, NSUB, 1150], bf16, tag="R")
        nc.vector.tensor_tensor(
            out=R[:, :, 0:HF].rearrange("p s (h f) -> p s h f", h=H),
            in0=S[:, :, 0:F].unsqueeze(2).broadcast_to([128, NSUB, H, F]),
            in1=exb[:].unsqueeze(3).broadcast_to([128, NSUB, H, F]),
            op=mybir.AluOpType.mult)
        nc.vector.tensor_copy(R[:, :, HF:1150], exb[:])

        y_ps = ps.tile([128, 1150], f32, tag="ps_big")
        for s in range(NSUB):
            for c0, c1 in ((0, 512), (512, 1024), (1024, 1150)):
                nc.tensor.matmul(y_ps[:, c0:c1], Mt[:, s, :], R[:, s, c0:c1],
                                 start=(s == 0), stop=(s == NSUB - 1))

        den = sb.tile([128, H], f32, tag="den")
        nc.vector.tensor_scalar(out=den[:], in0=y_ps[:, HF:1150], scalar1=inval[:],
                                scalar2=None, op0=mybir.AluOpType.add)
        rden = sb.tile([128, H], f32, tag="rden")
        nc.vector.reciprocal(rden[:], den[:])
        rden2 = sb.tile([128, H], bf16, tag="rden2")
        nc.vector.tensor_scalar(out=rden2[:], in0=rden[:], scalar1=dinv[:],
                                scalar2=None, op0=mybir.AluOpType.mult)

        yt = sb.tile([128, 1290], bf16, tag="yt")
        ytv = yt[:, 0:1280].rearrange("p (h c) -> p h c", h=H)
        ypv = y_ps[:, 0:HF].rearrange("p (h f) -> p h f", h=H)
        nc.vector.tensor_copy(ytv[:, 0:5, 0:F], ypv[:, 0:5, :])
        nc.scalar.copy(ytv[:, 5:H, 0:F], ypv[:, 5:H, :])
        nc.vector.tensor_copy(ytv[:, :, F:F + 1], den[:].unsqueeze(2))
        nc.vector.memset(ytv[:, :, F + 1:128], 0.0)
        nc.vector.tensor_copy(yt[:, 1280:1290], rden2[:])
        w = nc.sync.dma_start(y_dram[b * 128:(b + 1) * 128, :], yt[:])
        y_writes.append(w)

    # ---------------- Phase 3: GAT W-pass (z production) ----------------
    z_writes = []
    for t in range(NBLK):
        rdn = sb.tile([128, H], bf16, tag="rdn")
        r = nc.sync.dma_start(rdn[:], y_dram[t * 128:(t + 1) * 128, 1280:1290])
        add_dep_helper(r.ins, y_writes[t].ins, reason="rden after y write")
        zp = ps.tile([128, 1280], f32, tag="ps_big")
        for h in range(H):
            yT = sb.tile([128, 128], bf16, tag="yT")
            ld = nc.sync.dma_start_transpose(
                out=yT[:], in_=y_dram[t * 128:(t + 1) * 128, h * 128:(h + 1) * 128])
            add_dep_helper(ld.ins, y_writes[t].ins, reason="yT after y write")
            bank = (h * 128) // 512
            first = (h % 4 == 0)
            last = (h % 4 == 3) or (h == H - 1)
            nc.tensor.matmul(zp[:, h * 128:h * 128 + F], yT[:], gatWk[:, h, :],
                             start=first, stop=last)
        zt = sb.tile([128, 1152], bf16, tag="zt")
        zpre = sb.tile([128, HF], f32, tag="zpre")
        nc.vector.tensor_tensor(
            out=zpre[:].rearrange("p (h f) -> p h f", h=H),
            in0=zp[:].rearrange("p (h c) -> p h c", h=H)[:, :, 0:F],
            in1=rdn[:].unsqueeze(2).broadcast_to([128, H, F]),
            op=mybir.AluOpType.mult)
        nc.vector.scalar_tensor_tensor(out=zt[:, 0:HF], in0=zpre[:], scalar=0.01,
                                       in1=zpre[:], op0=mybir.AluOpType.mult,
                                       op1=mybir.AluOpType.max)
        nc.vector.memset(zt[:, HF:1152], 0.0)
        w = nc.sync.dma_start(z_loc[t * 128:(t + 1) * 128, :], zt[:])
        z_writes.append(w)
    ag_z = nc.gpsimd.collective_compute(
        "AllGather", mybir.AluOpType.bypass, replica_groups=rg,
        ins=[z_loc[:]], outs=[z_glob[:]])
    for w in z_writes:
        add_dep_helper(ag_z.ins, w.ins, reason="AG_z after z writes")

    return dict(st=st, isrc=isrc, maskD=maskD, dinvD=dinvD, validD=validD,
                z_glob=z_glob, ag_z=ag_z, pfx=pfx)


def _build_gcn(nc, tc, ctx, pools, br, pool_loc, pool_col0, plw):
    """GCN aggregation + W-pass + pooling for one branch."""
    st = br["st"]
    MAXG, SS, NBLK, NSUB = st["MAXG"], st["SS"], st["NBLK"], st["NSUB"]
    CAP = NSUB * 128
    pfx = br["pfx"]
    d = nc.dram_tensor
    gcnWkD = d(pfx + "gcnWk", [9, 128, 1152], bf16, kind="ExternalInput")
    poisonD = d(pfx + "poison", [128, SS], bf16, kind="ExternalInput")
    rcntD = d(pfx + "rcnt", [128, GPC], f32, kind="ExternalInput")
    y2_dram = d(pfx + "y2", [SS, 1152], bf16)

    sb, ps = pools["sb"], pools["ps"]

    # ---------------- Phase 4: GCN aggregation ----------------
    y2_writes = []
    for b in range(NBLK):
        ii = sb.tile([128, NSUB * 8], i16, tag="ii")
        nc.sync.dma_start(ii[:], br["isrc"][b])
        Z = sb.tile([128, NSUB, 1152], bf16, tag="Z")
        _gather_chunked(nc, pools, Z, br["z_glob"][:], ii, CAP, 1152,
                        deps=(br["ag_z"],))
        Mt = sb.tile([128, NSUB, 128], bf16, tag="Mt")
        nc.sync.dma_start(Mt[:], br["maskD"][b])
        dinv = sb.tile([128, 1], f32, tag="col2")
        nc.sync.dma_start(dinv[:], br["dinvD"][b])
        valid = sb.tile([128, 1], f32, tag="col")
        nc.sync.dma_start(valid[:], br["validD"][b])

        y2_ps = ps.tile([128, HF], f32, tag="ps_big")
        for s in range(NSUB):
            for c0, c1 in ((0, 512), (512, 1024), (1024, HF)):
                nc.tensor.matmul(y2_ps[:, c0:c1], Mt[:, s, :], Z[:, s, c0:c1],
                                 start=(s == 0), stop=(s == NSUB - 1))
        y2t = sb.tile([128, 1152], bf16, tag="zt")
        nc.vector.tensor_scalar(out=y2t[:, 0:512], in0=y2_ps[:, 0:512],
                                scalar1=dinv[:], scalar2=None,
                                op0=mybir.AluOpType.mult)
        nc.scalar.activation(y2t[:, 512:HF], y2_ps[:, 512:HF],
                             mybir.ActivationFunctionType.Copy, scale=dinv[:])
        nc.vector.tensor_copy(y2t[:, HF:HF + 1], valid[:])
        nc.vector.memset(y2t[:, HF + 1:1152], 0.0)
        w = nc.sync.dma_start(y2_dram[b * 128:(b + 1) * 128, :], y2t[:])
        y2_writes.append(w)

    # ---------------- Phase 5: GCN W-pass + pooling ----------------
    gcnWk = sb.tile([128, 9, 1152], bf16, tag="gcnwk", bufs=1)
    nc.sync.dma_start(gcnWk[:], gcnWkD.ap().rearrange("kb kr n -> kr kb n"))
    zfin = sb.tile([128, 9, SS], bf16, tag="zfin", bufs=1)
    groups = []
    r = 0
    while r < SS:
        groups.append((r, min(SS, r + 512)))
        r += 512
    for (r0, r1) in groups:
        gw = r1 - r0
        yTs = []
        for kb in range(9):
            y2T = sb.tile([128, 512], bf16, tag=f"y2T{kb}")
            ld = nc.sync.dma_start_transpose(
                out=y2T[:, 0:gw], in_=y2_dram[r0:r1, kb * 128:(kb + 1) * 128])
            for bb in range(r0 // 128, (r1 + 127) // 128):
                add_dep_helper(ld.ins, y2_writes[bb].ins, reason="y2T after y2 write")
            yTs.append(y2T)
        for nb in range(9):
            ct = ps.tile([128, 512], f32, tag="ps_small")
            for kb in range(9):
                nc.tensor.matmul(ct[:, 0:gw], gcnWk[:, kb, nb * 128:(nb + 1) * 128],
                                 yTs[kb][:, 0:gw], start=(kb == 0), stop=(kb == 8))
            nc.scalar.activation(zfin[:, nb, r0:r1], ct[:, 0:gw],
                                 mybir.ActivationFunctionType.Lrelu, alpha=0.01)

    # pooling
    poison = sb.tile([128, SS], bf16, tag="poison", bufs=1)
    nc.sync.dma_start(poison[:], poisonD[:])
    rcnt = sb.tile([128, GPC], f32, tag="rcnt")
    nc.sync.dma_start(rcnt[:], rcntD[:])
    mxT = sb.tile([128, 9, GPC], f32, tag="mxT")
    smT = sb.tile([128, 9, GPC], f32, tag="smT")
    for g in range(GPC):
        s0 = g * MAXG
        tmp = sb.tile([128, 9, MAXG], bf16, tag="ptmp")
        nc.vector.tensor_tensor(
            out=tmp[:], in0=zfin[:, :, s0:s0 + MAXG],
            in1=poison[:, s0:s0 + MAXG].unsqueeze(1).broadcast_to([128, 9, MAXG]),
            op=mybir.AluOpType.add)
        for ft in range(9):
            nc.vector.reduce_max(mxT[:, ft, g:g + 1], tmp[:, ft, :],
                                 axis=mybir.AxisListType.X)
            nc.vector.reduce_sum(smT[:, ft, g:g + 1], zfin[:, ft, s0:s0 + MAXG],
                                 axis=mybir.AxisListType.X)
    mnT = sb.tile([128, 9, GPC], f32, tag="mnT")
    nc.vector.tensor_tensor(out=mnT[:], in0=smT[:],
                            in1=rcnt[:].unsqueeze(1).broadcast_to([128, 9, GPC]),
                            op=mybir.AluOpType.mult)

    # stage into pool_loc[16, col0 : col0+2304]
    ident = pools["ident"]
    writes = []
    for which, statT in ((0, mxT), (1, mnT)):
        for ft in range(9):
            tp = ps.tile([GPC, 128], f32, tag="ps_small")
            nc.tensor.transpose(tp[:], statT[:, ft, :], ident[:])
            stg = sb.tile([GPC, 128], f32, tag="stg")
            nc.vector.tensor_copy(stg[:], tp[:])
            w = nc.sync.dma_start(
                pool_loc[:, pool_col0 + which * 1152 + ft * 128:
                         pool_col0 + which * 1152 + ft * 128 + 128], stg[:])
            add_dep_helper(w.ins, plw.ins, reason="stage after pool init")
            writes.append(w)
    return writes


def _build_tail(nc, tc, ctx, pools, pool_glob, ag_pool):
    d = nc.dram_tensor
    sb, ps = pools["sb"], pools["ps"]
    ident = pools["ident"]
    tgtD = d("target", [G, 1000], f32, kind="ExternalInput")
    fcxtWkD = d("fcxtWk", [8, 128, 128], f32, kind="ExternalInput")
    fc1WkD = d("fc1Wk", [3, 128, 128], f32, kind="ExternalInput")
    fc2WkD = d("fc2Wk", [2, 128, 32], f32, kind="ExternalInput")
    outWkD = d("outWk", [128, 1], f32, kind="ExternalInput")
    outD = d("out", [G, 1], f32, kind="ExternalOutput")

    def pe_T(src_ap, rows):
        tp = ps.tile([rows, 128], f32, tag="ps_small")
        nc.tensor.transpose(tp[:], src_ap, ident[:])
        return tp

    def mm_transposed(src_tile, nk, rhs_fn, psum, chunks, tag):
        """Accumulate psum += src^T-tile_k.T @ rhs_k, interleaving the PE
        transposes with the accumulation matmuls (same-engine slot safety)."""
        for k in range(nk):
            tp = pe_T(src_tile[:, k * 128:(k + 1) * 128], 128)
            tt = sb.tile([128, 128], f32, tag=tag)
            nc.vector.tensor_copy(tt[:], tp[:])
            for c0, c1 in chunks:
                nc.tensor.matmul(psum[:, c0:c1], tt[:], rhs_fn(k)[:, c0:c1],
                                 start=(k == 0), stop=(k == nk - 1))

    # xt = target @ fcxtW + b  (ones col at 1000)
    tg = sb.tile([128, 1024], f32, tag="tg")
    nc.sync.dma_start(tg[:, 0:1000], tgtD[:])
    nc.vector.memset(tg[:, 1000:1001], 1.0)
    nc.vector.memset(tg[:, 1001:1024], 0.0)
    fcxtWk = sb.tile([128, 8, 128], f32, tag="tw8")
    nc.sync.dma_start(fcxtWk[:], fcxtWkD.ap().rearrange("k r n -> r k n"))
    xt_ps = ps.tile([128, 128], f32, tag="ps_small")
    mm_transposed(tg, 8, lambda k: fcxtWk[:, k, :], xt_ps, ((0, 128),), "ttl")
    xt_sb = sb.tile([128, 128], f32, tag="xt")
    nc.vector.tensor_copy(xt_sb[:], xt_ps[:])

    # per-branch g vectors
    gvecs = []
    for bi, p in enumerate(("p1", "p2")):
        fg1D = d(p + "_fcg1Wk", [19, 128, 1024], f32, kind="ExternalInput")
        fg2D = d(p + "_fcg2Wk", [8, 128, 64], f32, kind="ExternalInput")
        fg1 = sb.tile([128, 19, 1024], f32, tag="fg1", bufs=1)
        nc.sync.dma_start(fg1[:], fg1D.ap().rearrange("k r n -> r k n"))
        g_ps = ps.tile([128, 1024], f32, tag="ps_big")
        kts = list(range(bi * 18, bi * 18 + 18)) + [36]
        for k, kt in enumerate(kts):
            pl0 = sb.tile([128, 128], f32, tag="pl0")
            ld = nc.sync.dma_start(pl0[:], pool_glob[:, kt * 128:(kt + 1) * 128])
            add_dep_helper(ld.ins, ag_pool.ins, reason="pool load after AG")
            tp = pe_T(pl0[:], 128)
            pl = sb.tile([128, 128], f32, tag="plt")
            nc.vector.tensor_copy(pl[:], tp[:])
            for c0, c1 in ((0, 512), (512, 1024)):
                nc.tensor.matmul(g_ps[:, c0:c1], pl[:], fg1[:, k, c0:c1],
                                 start=(k == 0), stop=(k == 18))
        glr = sb.tile([128, 1024], f32, tag="glr")
        nc.scalar.activation(glr[:, 0:1000], g_ps[:, 0:1000],
                             mybir.ActivationFunctionType.Lrelu, alpha=0.01)
        nc.vector.memset(glr[:, 1000:1001], 1.0)
        nc.vector.memset(glr[:, 1001:1024], 0.0)
        fg2 = sb.tile([128, 8, 64], f32, tag="tw8b")
        nc.sync.dma_start(fg2[:], fg2D.ap().rearrange("k r n -> r k n"))
        g2_ps = ps.tile([128, 64], f32, tag="ps_small")
        mm_transposed(glr, 8, lambda k: fg2[:, k, :], g2_ps, ((0, 64),), "gtl")
        gv = sb.tile([128, 64], f32, tag=f"gv{bi}")
        nc.vector.tensor_copy(gv[:], g2_ps[:])
        gvecs.append(gv)

    # xcT k-tiles
    xcT0 = sb.tile([128, 128], f32, tag="xcT0")
    t0 = pe_T(gvecs[0][:], 64)
    nc.vector.tensor_copy(xcT0[0:64, :], t0[:])
    t1 = pe_T(gvecs[1][:], 64)
    nc.vector.tensor_copy(xcT0[64:128, :], t1[:])
    xcT1 = sb.tile([128, 128], f32, tag="xcT1")
    t2 = pe_T(xt_sb[:], 128)
    nc.vector.tensor_copy(xcT1[:], t2[:])
    ones = sb.tile([128, 128], f32, tag="ones")
    nc.vector.memset(ones[:], 0.0)
    nc.vector.memset(ones[0:1, :], 1.0)

    fc1Wk = sb.tile([128, 3, 128], f32, tag="fc1w")
    nc.sync.dma_start(fc1Wk[:], fc1WkD.ap().rearrange("k r n -> r k n"))
    xc1_ps = ps.tile([128, 128], f32, tag="ps_small")
    for k, lt in enumerate((xcT0, xcT1, ones)):
        nc.tensor.matmul(xc1_ps[:], lt[:], fc1Wk[:, k, :], start=(k == 0), stop=(k == 2))
    xc1 = sb.tile([128, 128], f32, tag="xc1")
    nc.scalar.activation(xc1[:], xc1_ps[:],
                         mybir.ActivationFunctionType.Lrelu, alpha=0.01)
    xc1T = sb.tile([128, 128], f32, tag="xc1T")
    t3 = pe_T(xc1[:], 128)
    nc.vector.tensor_copy(xc1T[:], t3[:])

    fc2Wk = sb.tile([128, 2, 32], f32, tag="fc2w")
    nc.sync.dma_start(fc2Wk[:], fc2WkD.ap().rearrange("k r n -> r k n"))
    xc2_ps = ps.tile([128, 32], f32, tag="ps_small")
    for k, lt in enumerate((xc1T, ones)):
        nc.tensor.matmul(xc2_ps[:], lt[:], fc2Wk[:, k, :], start=(k == 0), stop=(k == 1))
    xc2 = sb.tile([128, 32], f32, tag="xc2")
    nc.scalar.activation(xc2[:], xc2_ps[:],
                         mybir.ActivationFunctionType.Lrelu, alpha=0.01)
    xc2T = sb.tile([128, 128], f32, tag="xc2T")
    nc.vector.memset(xc2T[:], 0.0)
    t4 = pe_T(xc2[:], 32)
    nc.vector.tensor_copy(xc2T[0:32, :], t4[:])
    nc.vector.memset(xc2T[32:33, :], 1.0)

    outWk = sb.tile([128, 1], f32, tag="outw")
    nc.sync.dma_start(outWk[:], outWkD[:])
    out_ps = ps.tile([128, 1], f32, tag="ps_small")
    nc.tensor.matmul(out_ps[:], xc2T[:], outWk[:], start=True, stop=True)
    outsb = sb.tile([128, 1], f32, tag="outsb")
    nc.vector.tensor_copy(outsb[:], out_ps[:])
    nc.sync.dma_start(outD[:], outsb[:])


def _build_program(st1, st2):
    nc = bacc.Bacc("TRN2", target_bir_lowering=False, debug=False, num_devices=NC)
    d = nc.dram_tensor
    identD = d("identity", [128, 128], f32, kind="ExternalInput")
    pool_loc = d("pool_loc", [GPC, 4736], f32)
    pool_glob = d("pool_glob", [G, 4736], f32, addr_space="Shared")

    with tile.TileContext(nc) as tc:
        with (
            tc.tile_pool(name="base", bufs=1) as base,
            tc.tile_pool(name="ps", bufs=2, space="PSUM") as ps,
        ):
            lib = nc.gpsimd.load_library(library_config.mlp)
            ident = base.tile([128, 128], f32, tag="ident")
            nc.sync.dma_start(ident[:], identD[:])
            identb = base.tile([128, 128], bf16, tag="identb")
            nc.vector.tensor_copy(identb[:], ident[:])
            pools = {"ps": ps, "ident": ident, "identb": identb, "lib": lib}
            ctx = None

            # zero the pooled staging buffer (cols 4608 bias=1, rest padded 0)
            stg0 = base.tile([GPC, 4736], f32, tag="stg0")
            nc.vector.memset(stg0[:], 0.0)
            nc.vector.memset(stg0[:, 4608:4609], 1.0)
            plw = nc.sync.dma_start(pool_loc[:], stg0[:])

            brs = []
            for pfx, st in (("b1_", st1), ("b2_", st2)):
                with tc.tile_pool(name="gat" + pfx, bufs=2) as sb:
                    pools["sb"] = sb
                    brs.append(_build_branch(nc, tc, ctx, pools, pfx, st, {}))
            ws = []
            for br, col0 in ((brs[0], 0), (brs[1], 2304)):
                with tc.tile_pool(name="gcn" + br["pfx"], bufs=2) as sb:
                    pools["sb"] = sb
                    ws.extend(_build_gcn(nc, tc, ctx, pools, br, pool_loc, col0, plw))
            ag_pool = nc.gpsimd.collective_compute(
                "AllGather", mybir.AluOpType.bypass,
                replica_groups=[list(range(NC))],
                ins=[pool_loc[:]], outs=[pool_glob[:]])
            add_dep_helper(ag_pool.ins, plw.ins, reason="AG pool after init")
            for w in ws:
                add_dep_helper(ag_pool.ins, w.ins, reason="AG pool after stage writes")
            with tc.tile_pool(name="tail", bufs=2) as sb:
                pools["sb"] = sb
                _build_tail(nc, tc, ctx, pools, pool_glob, ag_pool)

    nc.compile()
    return nc


# ---------------------------------------------------------------- entry point
def kernel(**inputs) -> np.ndarray:
    st1, pc1, sh1 = _prep_branch(inputs["x1"], inputs["edge_index1"], inputs["batch1"],
                                 inputs["p1_gatW"], inputs["p1_att_src"],
                                 inputs["p1_att_dst"])
    st2, pc2, sh2 = _prep_branch(inputs["x2"], inputs["edge_index2"], inputs["batch2"],
                                 inputs["p2_gatW"], inputs["p2_att_src"],
                                 inputs["p2_att_dst"])
    gatWk1, gcnWk1 = _pack_branch_weights(inputs["p1_gatW"], inputs["p1_gatb"],
                                          inputs["p1_gcnW"], inputs["p1_gcnb"])
    gatWk2, gcnWk2 = _pack_branch_weights(inputs["p2_gatW"], inputs["p2_gatb"],
                                          inputs["p2_gcnW"], inputs["p2_gcnb"])
    tail = _pack_tail(inputs)

    key = (st1["MAXG"], st1["NSUB"], st2["MAXG"], st2["NSUB"])
    if key not in _PROG_CACHE:
        _PROG_CACHE[key] = _build_program(st1, st2)
    nc = _PROG_CACHE[key]

    in_maps = []
    for c in range(NC):
        m = {"identity": tail["identity"], "target": tail["target"],
             "fcxtWk": tail["fcxtWk"], "fc1Wk": tail["fc1Wk"],
             "fc2Wk": tail["fc2Wk"], "outWk": tail["outWk"],
             "p1_fcg1Wk": tail["p1_fcg1Wk"], "p1_fcg2Wk": tail["p1_fcg2Wk"],
             "p2_fcg1Wk": tail["p2_fcg1Wk"], "p2_fcg2Wk": tail["p2_fcg2Wk"]}
        for pfx, pc, sh, gatWk, gcnWk in (("b1_", pc1, sh1, gatWk1, gcnWk1),
                                          ("b2_", pc2, sh2, gatWk2, gcnWk2)):
            p = pc[c]
            m[pfx + "xpad"] = sh["xpad"]
            m[pfx + "Wsd"] = sh["Wsd"]
            m[pfx + "gatWk"] = gatWk
            m[pfx + "gcnWk"] = gcnWk
            for k in ("isrc", "mask", "maskT", "xidx", "dinv", "valid", "inval",
                      "poison", "rcnt"):
                m[pfx + k] = p[k]
        in_maps.append(m)

    res = run_bass_kernel_spmd(nc, in_maps, list(range(NC)))
    return np.asarray(res.results[0]["out"], dtype=np.float32)



# revision 12
# speedup vs baseline: 1.5138x; 1.5138x over previous
"""Trainium2 Bass kernel for nn_GAT_GCN (GAT conv + GCN conv + pooling + MLP tail).

Strategy (8 NeuronCores, SPMD, full inputs in / full output out), v3:
  - Nodes live in graph-aligned "slots" (MAXG per graph, 16 graphs per core);
    core c owns graphs [16c, 16c+16).  Edges are sharded by destination node
    and bucketed per 128-slot destination block; a host-built one-hot mask
    turns the per-destination segment-sum into TensorEngine matmuls.
  - Per-node tables T = [x | a_src] and T2 = [a_dst] (256 B bf16 rows) are
    built REPLICATED on every core from sequential reads of x (no collective,
    no index gathers).  Per-edge payloads come from dma_gathers of T[src] and
    T2[dst]; gather lengths are trimmed to the per-block max edge count
    (rounded to 16) instead of a global pad.
  - GAT: exp(lrelu(a_s+a_d)) per edge; softmax + symmetric-norm scaling are
    folded into the aggregated y tile (scaled by dinv/den per head on the
    Scalar engine) before the head-blocked weight multiply, which uses
    half-branch-batched DMA-transpose loads.  z := lrelu(dinv * GAT_out).
  - z is AllGathered (bf16) once per branch; the GCN aggregation gathers
    z[src] rows as the mask-matmul rhs; dinv[dst] applied on output.  The
    dense 1152x1152 multiply runs as a transposed pass producing zfin^T,
    which feeds max/mean pooling via free-dim reductions.
  - Emission is interleaved across branches at block granularity so that
    every engine queue always holds independent work:  z-pass(b1) overlaps
    GAT-agg(b2), z-pass(b2) overlaps GCN-agg(b1), GCN-W(b1) overlaps
    GCN-agg(b2), and each branch's z-AllGather overlaps the other branch's
    compute.  The tiny MLP tail is replicated on every core.

Host-side preprocessing is restricted to index manipulation (sorting/
bucketing edges, one-hot masks, padding, row permutations) and parameter
repacking (padding / bf16 casts / tiny reshapes) -- all data-dependent float
compute runs on device.
"""

import numpy as np
import ml_dtypes

import concourse.bacc as bacc
import concourse.tile as tile
from concourse import mybir, library_config
from concourse.bass_utils import run_bass_kernel_spmd
from concourse.tile_rust import add_dep_helper

# ---------------------------------------------------------------- constants
N = 20000
E = 160000
G = 128
F = 114
H = 10
HF = 1140          # F * H
NC = 8
GPC = G // NC      # graphs per core
P = 128

bf16 = mybir.dt.bfloat16
f32 = mybir.dt.float32
i16 = mybir.dt.int16

BF = ml_dtypes.bfloat16

_PROG_CACHE: dict = {}


# ---------------------------------------------------------------- host utils
def _wrap_idx(idx: np.ndarray) -> np.ndarray:
    """int16 index list (len % 16 == 0) -> [128, len/16] wrapped layout."""
    n = idx.shape[0]
    assert n % 16 == 0
    return np.tile(idx.reshape(-1, 16).T, (8, 1)).astype(np.int16)


def _prep_branch(x, ei, batch, gatW, att_src, att_dst):
    """Host preprocessing for one branch. Returns (static, percore, shared)."""
    x = np.asarray(x, dtype=np.float32)
    ei = np.asarray(ei).astype(np.int64)
    batch = np.asarray(batch).astype(np.int64)
    gatW = np.asarray(gatW, dtype=np.float32)
    att_src = np.asarray(att_src, dtype=np.float32)
    att_dst = np.asarray(att_dst, dtype=np.float32)

    cnt = np.bincount(batch, minlength=G)
    MAXG = int(cnt.max())
    SS = ((16 * MAXG + 127) // 128) * 128       # slots per core shard
    NBLK = SS // 128
    assert NC * SS <= 32768, "row ids must fit int16"

    gstart = np.zeros(G + 1, np.int64)
    np.cumsum(cnt, out=gstart[1:])
    nodes = np.arange(N)
    rank = nodes - gstart[batch]
    slot_of_node = (batch % GPC) * MAXG + rank          # [N] in [0, 16*MAXG)
    core_of_node = batch // GPC                          # [N]
    row_of_node = core_of_node * SS + slot_of_node       # [N] global table row

    src = np.concatenate([ei[0], nodes])
    dst = np.concatenate([ei[1], nodes])

    core_e = core_of_node[dst]
    slot_e = slot_of_node[dst]
    blk_e = slot_e // 128
    dloc_e = slot_e % 128

    # bucket edges per (core, block)
    order = np.lexsort((blk_e, core_e))
    src_s = src[order]
    dst_s = dst[order]
    blk_s, dloc_s = blk_e[order], dloc_e[order]
    key = core_e[order] * NBLK + blk_s
    counts = np.bincount(key, minlength=NC * NBLK)
    starts = np.zeros(NC * NBLK + 1, np.int64)
    np.cumsum(counts, out=starts[1:])
    co = counts.reshape(NC, NBLK)
    # exact subtile / gather-length per block, shared across cores
    maxcnt = co.max(axis=0)
    NSUBS = tuple(int(v) for v in np.maximum(1, -(-maxcnt // 128)))
    NGS = tuple(int(v) for v in np.maximum(16, 16 * (-(-maxcnt // 16))))
    TOTSUB = sum(NSUBS)
    TOTNG = sum(NGS)
    NSUBMAX = max(NSUBS)

    deg = np.bincount(dst, minlength=N).astype(np.float64)
    dinv_node = 1.0 / np.sqrt(np.maximum(deg, 1.0))

    slots = np.arange(SS)
    g_local = np.minimum(slots // MAXG, GPC - 1)
    r_local = slots - g_local * MAXG

    ngoff = np.zeros(NBLK + 1, np.int64)
    np.cumsum(NGS, out=ngoff[1:])

    percore = []
    for c in range(NC):
        isrc = np.zeros(TOTNG, np.int64)
        idst = np.zeros(TOTNG, np.int64)
        mask = np.zeros((128, TOTSUB, 128), np.float32)   # [e, (b s), d]
        soff = 0
        for b in range(NBLK):
            k = c * NBLK + b
            cnt_b = int(counts[k])
            sl = slice(starts[k], starts[k + 1])
            o = ngoff[b]
            isrc[o:o + cnt_b] = row_of_node[src_s[sl]]
            idst[o:o + cnt_b] = row_of_node[dst_s[sl]]
            pos = np.arange(cnt_b)
            mask[pos % 128, soff + pos // 128, dloc_s[sl]] = 1.0
            soff += NSUBS[b]

        isrc_w = np.concatenate(
            [_wrap_idx(isrc[ngoff[b]:ngoff[b + 1]].astype(np.int16))
             for b in range(NBLK)], axis=1)
        idst_w = np.concatenate(
            [_wrap_idx(idst[ngoff[b]:ngoff[b + 1]].astype(np.int16))
             for b in range(NBLK)], axis=1)

        g_global = c * GPC + g_local
        validity = (slots < 16 * MAXG) & (r_local < cnt[g_global])
        node_of_slot = np.zeros(SS, np.int64)
        real = validity.nonzero()[0]
        node_of_slot[real] = gstart[g_global[real]] + r_local[real]

        dinv_slot = np.ones(SS, np.float32)
        dinv_slot[real] = dinv_node[node_of_slot[real]]
        valid = validity.astype(np.float32)

        cols = np.zeros((128, NBLK, 4), np.float32)
        cols[:, :, 0] = dinv_slot.reshape(NBLK, 128).T
        cols[:, :, 1] = (1.0 - valid).reshape(NBLK, 128).T
        cols[:, :, 2] = valid.reshape(NBLK, 128).T

        poison = np.where(validity, 0.0, -1e28).astype(np.float32)
        rcnt = (1.0 / np.maximum(cnt[c * GPC:(c + 1) * GPC], 1.0)).astype(np.float32)

        percore.append({
            "isrc": isrc_w,                                          # [128, TOTNG/16]
            "idst": idst_w,
            "mask": mask.astype(BF),                                 # [128, TOTSUB, 128]
            "cols": cols,                                            # [128, NBLK, 4]
            "poison": np.broadcast_to(poison.astype(BF), (128, SS)).copy(),
            "rcnt": np.broadcast_to(rcnt, (128, GPC)).copy(),
        })

    # shared tables: slot-ordered x (replicated T build), plus its transpose
    xslot = np.zeros((NC * SS, 128), np.float32)
    for c in range(NC):
        g_global = c * GPC + g_local
        validity = (slots < 16 * MAXG) & (r_local < cnt[g_global])
        real = validity.nonzero()[0]
        node = gstart[g_global[real]] + r_local[real]
        xslot[c * SS + real, :F] = x[node]

    gatW3 = gatW.reshape(F, H, F)
    Ws = np.einsum("khf,hf->kh", gatW3, att_src)
    Wd = np.einsum("khf,hf->kh", gatW3, att_dst)
    Wsd = np.zeros((128, 2 * H), np.float32)
    Wsd[:F, :H] = Ws
    Wsd[:F, H:] = Wd

    shared = {
        "xslot": xslot.astype(BF),                          # [NC*SS, 128]
        "xslotT": np.ascontiguousarray(xslot.T).astype(BF),  # [128, NC*SS]
        "Wsd": Wsd.astype(BF),                              # [128, 20]
    }
    static = dict(MAXG=MAXG, SS=SS, NBLK=NBLK, NSUBS=NSUBS, NGS=NGS,
                  TOTSUB=TOTSUB, TOTNG=TOTNG, NSUBMAX=NSUBMAX)
    return static, percore, shared


def _pack_branch_weights(gatW, gatb, gcnW, gcnb):
    gatW = np.asarray(gatW, np.float32)
    gatb = np.asarray(gatb, np.float32)
    gcnW = np.asarray(gcnW, np.float32)
    gcnb = np.asarray(gcnb, np.float32)
    gatWk = np.zeros((H, 128, F), np.float32)
    gatW3 = gatW.reshape(F, H, F)
    for h in range(H):
        gatWk[h, :F, :] = gatW3[:, h, :]
        gatWk[h, F, :] = gatb[h * F:(h + 1) * F]
    gcn_pad = np.zeros((1152, 1152), np.float32)
    gcn_pad[:HF, :HF] = gcnW
    gcn_pad[HF, :HF] = gcnb
    gcnWk = gcn_pad.reshape(9, 128, 1152)
    return gatWk.astype(BF), gcnWk.astype(BF)


def _pack_tail(inp):
    f = lambda k: np.asarray(inp[k], np.float32)
    tail = {}
    for p in ("p1", "p2"):
        W1 = np.zeros((2432, 1024), np.float32)
        fg1 = f(p + "_fcg1W")           # [2280, 1000]
        W1[0:HF, 0:1000] = fg1[0:HF]
        W1[1152:1152 + HF, 0:1000] = fg1[HF:2 * HF]
        k1 = np.concatenate([W1[:2304].reshape(18, 128, 1024),
                             np.zeros((1, 128, 1024), np.float32)], axis=0)
        k1[18, 0, 0:1000] = f(p + "_fcg1b")
        tail[p + "_fcg1Wk"] = k1.astype(np.float32)
        W2 = np.zeros((1024, 64), np.float32)
        W2[0:1000] = f(p + "_fcg2W")
        W2[1000] = f(p + "_fcg2b")
        tail[p + "_fcg2Wk"] = W2.reshape(8, 128, 64).astype(np.float32)
    Wx = np.zeros((1024, 128), np.float32)
    Wx[0:1000] = f("fcxtW")
    Wx[1000] = f("fcxtb")
    tail["fcxtWk"] = Wx.reshape(8, 128, 128).astype(np.float32)
    W1 = np.zeros((3, 128, 128), np.float32)
    W1[0] = f("fc1W")[0:128]
    W1[1] = f("fc1W")[128:256]
    W1[2, 0] = f("fc1b")
    tail["fc1Wk"] = W1.astype(np.float32)
    W2 = np.zeros((2, 128, 32), np.float32)
    W2[0] = f("fc2W")
    W2[1, 0] = f("fc2b")
    tail["fc2Wk"] = W2.astype(np.float32)
    Wo = np.zeros((128, 1), np.float32)
    Wo[0:32, 0] = f("outW")[:, 0]
    Wo[32, 0] = float(np.asarray(inp["outb"]).reshape(-1)[0])
    tail["outWk"] = Wo.astype(np.float32)
    tail["identity"] = np.eye(128, dtype=np.float32)
    tail["target"] = f("target")
    return tail


# ---------------------------------------------------------------- device build
GATHER_MAX = 1024  # dma_gather breaks above 1024 indices per call


class Branch:
    """Per-branch emission state: DRAM handles, resident tiles, generators."""

    def __init__(self, nc, pools, pfx, st):
        self.nc = nc
        self.pools = pools
        self.pfx = pfx
        self.st = st
        MAXG, SS, NBLK = st["MAXG"], st["SS"], st["NBLK"]
        TOTSUB, TOTNG = st["TOTSUB"], st["TOTNG"]
        d = nc.dram_tensor
        self.xslotD = d(pfx + "xslot", [NC * SS, 128], bf16, kind="ExternalInput")
        self.xslotTD = d(pfx + "xslotT", [128, NC * SS], bf16, kind="ExternalInput")
        self.WsdD = d(pfx + "Wsd", [128, 2 * H], bf16, kind="ExternalInput")
        self.isrcD = d(pfx + "isrc", [128, TOTNG // 16], i16, kind="ExternalInput")
        self.idstD = d(pfx + "idst", [128, TOTNG // 16], i16, kind="ExternalInput")
        self.maskD = d(pfx + "mask", [128, TOTSUB, 128], bf16, kind="ExternalInput")
        self.colsD = d(pfx + "cols", [128, NBLK, 4], f32, kind="ExternalInput")
        self.gatWkD = d(pfx + "gatWk", [H, 128, F], bf16, kind="ExternalInput")
        self.gcnWkD = d(pfx + "gcnWk", [9, 128, 1152], bf16, kind="ExternalInput")
        self.poisonD = d(pfx + "poison", [128, SS], bf16, kind="ExternalInput")
        self.rcntD = d(pfx + "rcnt", [128, GPC], f32, kind="ExternalInput")
        self.TD = d(pfx + "T", [NC * SS, 128], bf16)
        self.T2D = d(pfx + "T2", [NC * SS, 128], bf16)
        self.y_dram = d(pfx + "y", [SS, 1280], bf16)
        self.z_loc = d(pfx + "z_loc", [SS, 1152], bf16)
        self.z_glob = d(pfx + "z_glob", [NC * SS, 1152], bf16, addr_space="Shared")
        self.y2_dram = d(pfx + "y2", [SS, 1152], bf16)

        self.fence = None
        self.y_writes = []
        self.z_writes = []
        self.y2_writes = []
        self.ag_z = None

        # offsets
        self.soffs = np.zeros(NBLK + 1, np.int64)
        np.cumsum(st["NSUBS"], out=self.soffs[1:])
        self.ioffs = np.zeros(NBLK + 1, np.int64)
        np.cumsum([n // 16 for n in st["NGS"]], out=self.ioffs[1:])

    # ---- resident tiles (call once inside the long-lived base pool) ----
    def load_residents(self, base):
        nc = self.nc
        self.gatWk = base.tile([128, H, F], bf16, tag=self.pfx + "gatwk")
        nc.sync.dma_start(self.gatWk[:], self.gatWkD.ap().rearrange("h k n -> k h n"))
        self.cols = base.tile([128, self.st["NBLK"], 4], f32, tag=self.pfx + "cols")
        nc.sync.dma_start(self.cols[:], self.colsD[:])
        self.ii = base.tile([128, self.st["TOTNG"] // 16], i16, tag=self.pfx + "ii")
        nc.sync.dma_start(self.ii[:], self.isrcD[:])
        self.Wsd = base.tile([128, 2 * H], bf16, tag=self.pfx + "wsd")
        nc.sync.dma_start(self.Wsd[:], self.WsdD[:])

    def load_jj(self, sb):
        nc = self.nc
        self.jj = sb.tile([128, self.st["TOTNG"] // 16], i16,
                          tag=self.pfx + "jj", bufs=1)
        nc.sync.dma_start(self.jj[:], self.idstD[:])

    def _gather(self, out_tile, table_ap, idx_tile, b, elem, deps):
        nc, pools = self.nc, self.pools
        ng = self.st["NGS"][b]
        i0 = int(self.ioffs[b])
        if ng % 128:
            # pre-zero the partial last subtile; the gather then overwrites
            # rows [0, ng%128) of it (DVE memsets need aligned partition 0)
            nc.vector.memset(out_tile[:, ng // 128, :], 0.0)
        insts = []
        for o in range(0, ng, GATHER_MAX):
            n = min(ng, o + GATHER_MAX) - o
            g = nc.gpsimd.dma_gather(
                out_tile[:, o // 128:(o + n + 127) // 128, :], table_ap,
                idx_tile[:, i0 + o // 16:i0 + (o + n) // 16], n, n, elem)
            add_dep_helper(g.ins, pools["lib"].ins, reason="gather after lib")
            for dd in deps:
                add_dep_helper(g.ins, dd.ins, reason="gather dep")
            insts.append(g)
        return insts

    # ---- T build ----
    def emit_tbuild(self, sb):
        nc, ps = self.nc, self.pools["ps"]
        writes = []
        NTI = NC * self.st["SS"] // 1024
        for it in range(NTI):
            r0 = it * 1024
            xt = sb.tile([128, 1024], bf16, tag="xt")
            nc.sync.dma_start(xt[:], self.xslotTD[:, r0:r0 + 1024])
            xr = sb.tile([128, 8, 128], bf16, tag="xr")
            nc.sync.dma_start(
                xr[:], self.xslotD[r0:r0 + 1024, :].rearrange("(a p) c -> p a c", p=128))
            aps = ps.tile([128, 8, 2 * H], f32, tag="ps_small")
            for a in range(8):
                nc.tensor.matmul(aps[:, a, :], xt[:, a * 128:(a + 1) * 128],
                                 self.Wsd[:], start=True, stop=True)
            Tt = sb.tile([128, 8, 128], bf16, tag="Tt")
            nc.vector.tensor_copy(Tt[:, :, 0:F], xr[:, :, 0:F])
            nc.vector.tensor_copy(Tt[:, :, F:F + H], aps[:, :, 0:H])
            nc.vector.memset(Tt[:, :, F + H:128], 0.0)
            T2t = sb.tile([128, 8, 128], bf16, tag="T2t")
            nc.scalar.copy(T2t[:, :, 0:H], aps[:, :, H:2 * H])
            nc.vector.memset(T2t[:, :, H:128], 0.0)
            w1 = nc.sync.dma_start(
                self.TD[r0:r0 + 1024, :].rearrange("(a p) c -> p a c", p=128), Tt[:])
            w2 = nc.sync.dma_start(
                self.T2D[r0:r0 + 1024, :].rearrange("(a p) c -> p a c", p=128), T2t[:])
            writes.extend((w1, w2))
        fence_t = sb.tile([128, 1], f32, tag="fence" + self.pfx)
        self.fence = nc.vector.memset(fence_t[:], 0.0)
        for w in writes:
            add_dep_helper(self.fence.ins, w.ins, reason="T fence")

    # ---- GAT aggregation, one block ----
    def prezero_gat(self, sb):
        nc = self.nc
        NSM = self.st["NSUBMAX"]
        for _ in range(2):
            S = sb.tile([128, NSM, 128], bf16, tag="S")
            nc.vector.memset(S[:], 0.0)
            D = sb.tile([128, NSM, 128], bf16, tag="D")
            nc.vector.memset(D[:], 0.0)

    def emit_gat_block(self, sb, b):
        nc, ps = self.nc, self.pools["ps"]
        ns = self.st["NSUBS"][b]
        NSM = self.st["NSUBMAX"]
        soff = int(self.soffs[b])
        cols = self.cols

        S = sb.tile([128, NSM, 128], bf16, tag="S")
        self._gather(S, self.TD[:], self.ii, b, 128, (self.fence,))
        D = sb.tile([128, NSM, 128], bf16, tag="D")
        self._gather(D, self.T2D[:], self.jj, b, 128, (self.fence,))
        Mt = sb.tile([128, NSM, 128], bf16, tag="Mt")
        nc.sync.dma_start(Mt[:, 0:ns, :], self.maskD[:, soff:soff + ns, :])

        lg = sb.tile([128, NSM, H], f32, tag="lg")
        nc.vector.tensor_tensor(out=lg[:, 0:ns, :], in0=S[:, 0:ns, F:F + H],
                                in1=D[:, 0:ns, 0:H], op=mybir.AluOpType.add)
        l3 = sb.tile([128, NSM, H], f32, tag="l3")
        nc.vector.scalar_tensor_tensor(out=l3[:, 0:ns, :], in0=lg[:, 0:ns, :],
                                       scalar=0.2, in1=lg[:, 0:ns, :],
                                       op0=mybir.AluOpType.mult,
                                       op1=mybir.AluOpType.max)
        exb = sb.tile([128, NSM, H], bf16, tag="exb")
        nc.scalar.activation(exb[:, 0:ns, :], l3[:, 0:ns, :],
                             mybir.ActivationFunctionType.Exp)

        R = sb.tile([128, NSM, 1152], bf16, tag="R")
        nc.vector.tensor_tensor(
            out=R[:, 0:ns, 0:HF].rearrange("p s (h f) -> p s h f", h=H),
            in0=S[:, 0:ns, 0:F].unsqueeze(2).broadcast_to([128, ns, H, F]),
            in1=exb[:, 0:ns, :].unsqueeze(3).broadcast_to([128, ns, H, F]),
            op=mybir.AluOpType.mult)
        nc.scalar.copy(R[:, 0:ns, HF:1150], exb[:, 0:ns, :])

        y_ps = ps.tile([128, 1280], f32, tag="ps_big")
        for s in range(ns):
            for c0, c1 in ((0, 512), (512, 1024), (1024, 1150)):
                nc.tensor.matmul(y_ps[:, c0:c1], Mt[:, s, :], R[:, s, c0:c1],
                                 start=(s == 0), stop=(s == ns - 1))

        den = sb.tile([128, H], f32, tag="den")
        nc.vector.tensor_scalar(out=den[:], in0=y_ps[:, HF:1150],
                                scalar1=cols[:, b, 1:2], scalar2=None,
                                op0=mybir.AluOpType.add)
        rden = sb.tile([128, H], f32, tag="rden")
        nc.vector.reciprocal(rden[:], den[:])
        rdn = sb.tile([128, H], f32, tag="rdn")
        nc.vector.tensor_scalar(out=rdn[:], in0=rden[:], scalar1=cols[:, b, 0:1],
                                scalar2=None, op0=mybir.AluOpType.mult)

        y_sb = sb.tile([128, 1280], bf16, tag="ysb")
        ytv = y_sb[:].rearrange("p (h c) -> p h c", h=H)
        ypv = y_ps[:, 0:HF].rearrange("p (h f) -> p h f", h=H)
        for h in range(H):
            nc.scalar.activation(ytv[:, h, 0:F], ypv[:, h, :],
                                 mybir.ActivationFunctionType.Copy,
                                 scale=rdn[:, h:h + 1])
        nc.vector.tensor_copy(
            ytv[:, :, F:F + 1],
            cols[:, b, 0:1].unsqueeze(1).broadcast_to([128, H, 1]))
        nc.vector.memset(ytv[:, :, F + 1:128], 0.0)
        w = nc.sync.dma_start(self.y_dram[b * 128:(b + 1) * 128, :], y_sb[:])
        self.y_writes.append(w)

    # ---- z production: one half (list of blocks), transposes batched ----
    def emit_zpass_half(self, sb, blocks):
        nc, ps = self.nc, self.pools["ps"]
        t0, t1 = blocks[0], blocks[-1] + 1
        rows = (t1 - t0) * 128
        yTs = []
        for h in range(H):
            yT = sb.tile([128, 13 * 128], bf16, tag=f"yT{h}", bufs=1)
            ld = nc.sync.dma_start_transpose(
                out=yT[:, 0:rows],
                in_=self.y_dram[t0 * 128:t1 * 128, h * 128:(h + 1) * 128])
            for t in blocks:
                add_dep_helper(ld.ins, self.y_writes[t].ins, reason="yT after y")
            yTs.append(yT)
        for t in blocks:
            o = (t - t0) * 128
            zp = ps.tile([128, 1280], f32, tag="ps_big")
            for h in range(H):
                nc.tensor.matmul(zp[:, h * 128:h * 128 + F],
                                 yTs[h][:, o:o + 128], self.gatWk[:, h, :],
                                 start=(h % 4 == 0), stop=(h % 4 == 3) or (h == H - 1))
            zt = sb.tile([128, 1152], bf16, tag="zt")
            nc.scalar.activation(
                zt[:, 0:HF].rearrange("p (h f) -> p h f", h=H),
                zp[:].rearrange("p (h c) -> p h c", h=H)[:, :, 0:F],
                mybir.ActivationFunctionType.Lrelu, alpha=0.01)
            nc.vector.memset(zt[:, HF:1152], 0.0)
            w = nc.sync.dma_start(self.z_loc[t * 128:(t + 1) * 128, :], zt[:])
            self.z_writes.append(w)

    def emit_ag_z(self):
        nc = self.nc
        self.ag_z = nc.gpsimd.collective_compute(
            "AllGather", mybir.AluOpType.bypass,
            replica_groups=[list(range(NC))],
            ins=[self.z_loc[:]], outs=[self.z_glob[:]])
        for w in self.z_writes:
            add_dep_helper(self.ag_z.ins, w.ins, reason="AG_z after z writes")

    # ---- GCN aggregation, one block ----
    def prezero_gcn(self, sb):
        nc = self.nc
        NSM = self.st["NSUBMAX"]
        for _ in range(2):
            Z = sb.tile([128, NSM, 1152], bf16, tag="Z")
            nc.vector.memset(Z[:], 0.0)

    def emit_gcn_block(self, sb, b):
        nc, ps = self.nc, self.pools["ps"]
        ns = self.st["NSUBS"][b]
        NSM = self.st["NSUBMAX"]
        soff = int(self.soffs[b])
        cols = self.cols

        Z = sb.tile([128, NSM, 1152], bf16, tag="Z")
        self._gather(Z, self.z_glob[:], self.ii, b, 1152, (self.ag_z,))
        Mt = sb.tile([128, NSM, 128], bf16, tag="Mt2")
        nc.sync.dma_start(Mt[:, 0:ns, :], self.maskD[:, soff:soff + ns, :])

        y2_ps = ps.tile([128, 1280], f32, tag="ps_big")
        for s in range(ns):
            for c0, c1 in ((0, 512), (512, 1024), (1024, HF)):
                nc.tensor.matmul(y2_ps[:, c0:c1], Mt[:, s, :], Z[:, s, c0:c1],
                                 start=(s == 0), stop=(s == ns - 1))
        y2t = sb.tile([128, 1152], bf16, tag="y2t")
        nc.scalar.activation(y2t[:, 0:512], y2_ps[:, 0:512],
                             mybir.ActivationFunctionType.Copy,
                             scale=cols[:, b, 0:1])
        nc.scalar.activation(y2t[:, 512:HF], y2_ps[:, 512:HF],
                             mybir.ActivationFunctionType.Copy,
                             scale=cols[:, b, 0:1])
        nc.vector.tensor_copy(y2t[:, HF:HF + 1], cols[:, b, 2:3])
        nc.vector.memset(y2t[:, HF + 1:1152], 0.0)
        w = nc.sync.dma_start(self.y2_dram[b * 128:(b + 1) * 128, :], y2t[:])
        self.y2_writes.append(w)

    # ---- GCN W-pass: one half (transposes batched), writes zfin slices ----
    def alloc_zfin(self, sb):
        nc = self.nc
        self.gcnWk = sb.tile([128, 9, 1152], bf16, tag=self.pfx + "gcnwk", bufs=1)
        nc.sync.dma_start(self.gcnWk[:],
                          self.gcnWkD.ap().rearrange("kb kr n -> kr kb n"))
        self.zfin = sb.tile([128, 9, self.st["SS"]], bf16,
                            tag=self.pfx + "zfin", bufs=1)

    def emit_gcnw_half(self, sb, blocks):
        nc, ps = self.nc, self.pools["ps"]
        t0, t1 = blocks[0], blocks[-1] + 1
        rows = (t1 - t0) * 128
        r0 = t0 * 128
        yTs = []
        for kb in range(9):
            y2T = sb.tile([128, 13 * 128], bf16, tag=f"y2T{kb}", bufs=1)
            ld = nc.sync.dma_start_transpose(
                out=y2T[:, 0:rows],
                in_=self.y2_dram[r0:r0 + rows, kb * 128:(kb + 1) * 128])
            for t in blocks:
                add_dep_helper(ld.ins, self.y2_writes[t].ins, reason="y2T dep")
            yTs.append(y2T)
        for c0 in range(0, rows, 512):
            cw = min(512, rows - c0)
            for nb in range(9):
                ct = ps.tile([128, 512], f32, tag="ps_small")
                for kb in range(9):
                    nc.tensor.matmul(ct[:, 0:cw],
                                     self.gcnWk[:, kb, nb * 128:(nb + 1) * 128],
                                     yTs[kb][:, c0:c0 + cw],
                                     start=(kb == 0), stop=(kb == 8))
                nc.scalar.activation(self.zfin[:, nb, r0 + c0:r0 + c0 + cw],
                                     ct[:, 0:cw],
                                     mybir.ActivationFunctionType.Lrelu, alpha=0.01)

    # ---- pooling + staging ----
    def emit_pooling(self, sb, pool_loc, pool_col0, plw):
        nc, ps = self.nc, self.pools["ps"]
        MAXG = self.st["MAXG"]
        ident = self.pools["ident"]
        poison = sb.tile([128, self.st["SS"]], bf16, tag=self.pfx + "poison", bufs=1)
        nc.sync.dma_start(poison[:], self.poisonD[:])
        rcnt = sb.tile([128, GPC], f32, tag=self.pfx + "rcnt")
        nc.sync.dma_start(rcnt[:], self.rcntD[:])
        mxT = sb.tile([128, 9, GPC], f32, tag=self.pfx + "mxT")
        smT = sb.tile([128, 9, GPC], f32, tag=self.pfx + "smT")
        for g in range(GPC):
            s0 = g * MAXG
            tmp = sb.tile([128, 9, MAXG], bf16, tag="ptmp")
            nc.vector.tensor_tensor(
                out=tmp[:], in0=self.zfin[:, :, s0:s0 + MAXG],
                in1=poison[:, s0:s0 + MAXG].unsqueeze(1).broadcast_to([128, 9, MAXG]),
                op=mybir.AluOpType.add)
            nc.vector.reduce_max(mxT[:, :, g:g + 1], tmp[:],
                                 axis=mybir.AxisListType.X)
            nc.vector.reduce_sum(smT[:, :, g:g + 1], self.zfin[:, :, s0:s0 + MAXG],
                                 axis=mybir.AxisListType.X)
        mnT = sb.tile([128, 9, GPC], f32, tag=self.pfx + "mnT")
        nc.vector.tensor_tensor(out=mnT[:], in0=smT[:],
                                in1=rcnt[:].unsqueeze(1).broadcast_to([128, 9, GPC]),
                                op=mybir.AluOpType.mult)
        writes = []
        for which, statT in ((0, mxT), (1, mnT)):
            for ft in range(9):
                tp = ps.tile([GPC, 128], f32, tag="ps_small")
                nc.tensor.transpose(tp[:], statT[:, ft, :], ident[:])
                stg = sb.tile([GPC, 128], f32, tag="stg")
                nc.vector.tensor_copy(stg[:], tp[:])
                w = nc.sync.dma_start(
                    pool_loc[:, pool_col0 + which * 1152 + ft * 128:
                             pool_col0 + which * 1152 + ft * 128 + 128], stg[:])
                add_dep_helper(w.ins, plw.ins, reason="stage after pool init")
                writes.append(w)
        return writes


def _halves(NBLK):
    h = (NBLK + 1) // 2
    return [list(range(0, h)), list(range(h, NBLK))]


def _build_tail(nc, pools, pool_glob, ag_pool):
    d = nc.dram_tensor
    sb, ps = pools["sb"], pools["ps"]
    ident = pools["ident"]
    tgtD = d("target", [G, 1000], f32, kind="ExternalInput")
    fcxtWkD = d("fcxtWk", [8, 128, 128], f32, kind="ExternalInput")
    fc1WkD = d("fc1Wk", [3, 128, 128], f32, kind="ExternalInput")
    fc2WkD = d("fc2Wk", [2, 128, 32], f32, kind="ExternalInput")
    outWkD = d("outWk", [128, 1], f32, kind="ExternalInput")
    outD = d("out", [G, 1], f32, kind="ExternalOutput")

    def pe_T(src_ap, rows):
        tp = ps.tile([rows, 128], f32, tag="ps_small")
        nc.tensor.transpose(tp[:], src_ap, ident[:])
        return tp

    def mm_transposed(src_tile, nk, rhs_fn, psum, chunks, tag):
        for k in range(nk):
            tp = pe_T(src_tile[:, k * 128:(k + 1) * 128], 128)
            tt = sb.tile([128, 128], f32, tag=tag)
            nc.vector.tensor_copy(tt[:], tp[:])
            for c0, c1 in chunks:
                nc.tensor.matmul(psum[:, c0:c1], tt[:], rhs_fn(k)[:, c0:c1],
                                 start=(k == 0), stop=(k == nk - 1))

    tg = sb.tile([128, 1024], f32, tag="tg")
    nc.sync.dma_start(tg[:, 0:1000], tgtD[:])
    nc.vector.memset(tg[:, 1000:1001], 1.0)
    nc.vector.memset(tg[:, 1001:1024], 0.0)
    fcxtWk = sb.tile([128, 8, 128], f32, tag="tw8")
    nc.sync.dma_start(fcxtWk[:], fcxtWkD.ap().rearrange("k r n -> r k n"))
    xt_ps = ps.tile([128, 128], f32, tag="ps_small")
    mm_transposed(tg, 8, lambda k: fcxtWk[:, k, :], xt_ps, ((0, 128),), "ttl")
    xt_sb = sb.tile([128, 128], f32, tag="xt2")
    nc.vector.tensor_copy(xt_sb[:], xt_ps[:])

    gvecs = []
    for bi, p in enumerate(("p1", "p2")):
        fg1D = d(p + "_fcg1Wk", [19, 128, 1024], f32, kind="ExternalInput")
        fg2D = d(p + "_fcg2Wk", [8, 128, 64], f32, kind="ExternalInput")
        fg1 = sb.tile([128, 19, 1024], f32, tag="fg1", bufs=1)
        nc.sync.dma_start(fg1[:], fg1D.ap().rearrange("k r n -> r k n"))
        g_ps = ps.tile([128, 1024], f32, tag="ps_big")
        kts = list(range(bi * 18, bi * 18 + 18)) + [36]
        for k, kt in enumerate(kts):
            pl0 = sb.tile([128, 128], f32, tag="pl0")
            ld = nc.sync.dma_start(pl0[:], pool_glob[:, kt * 128:(kt + 1) * 128])
            add_dep_helper(ld.ins, ag_pool.ins, reason="pool load after AG")
            tp = pe_T(pl0[:], 128)
            pl = sb.tile([128, 128], f32, tag="plt")
            nc.vector.tensor_copy(pl[:], tp[:])
            for c0, c1 in ((0, 512), (512, 1024)):
                nc.tensor.matmul(g_ps[:, c0:c1], pl[:], fg1[:, k, c0:c1],
                                 start=(k == 0), stop=(k == 18))
        glr = sb.tile([128, 1024], f32, tag="glr")
        nc.scalar.activation(glr[:, 0:1000], g_ps[:, 0:1000],
                             mybir.ActivationFunctionType.Lrelu, alpha=0.01)
        nc.vector.memset(glr[:, 1000:1001], 1.0)
        nc.vector.memset(glr[:, 1001:1024], 0.0)
        fg2 = sb.tile([128, 8, 64], f32, tag="tw8b")
        nc.sync.dma_start(fg2[:], fg2D.ap().rearrange("k r n -> r k n"))
        g2_ps = ps.tile([128, 64], f32, tag="ps_small")
        mm_transposed(glr, 8, lambda k: fg2[:, k, :], g2_ps, ((0, 64),), "gtl")
        gv = sb.tile([128, 64], f32, tag=f"gv{bi}")
        nc.vector.tensor_copy(gv[:], g2_ps[:])
        gvecs.append(gv)

    xcT0 = sb.tile([128, 128], f32, tag="xcT0")
    t0 = pe_T(gvecs[0][:], 64)
    nc.vector.tensor_copy(xcT0[0:64, :], t0[:])
    t1 = pe_T(gvecs[1][:], 64)
    nc.vector.tensor_copy(xcT0[64:128, :], t1[:])
    xcT1 = sb.tile([128, 128], f32, tag="xcT1")
    t2 = pe_T(xt_sb[:], 128)
    nc.vector.tensor_copy(xcT1[:], t2[:])
    ones = sb.tile([128, 128], f32, tag="ones")
    nc.vector.memset(ones[:], 0.0)
    nc.vector.memset(ones[0:1, :], 1.0)

    fc1Wk = sb.tile([128, 3, 128], f32, tag="fc1w")
    nc.sync.dma_start(fc1Wk[:], fc1WkD.ap().rearrange("k r n -> r k n"))
    xc1_ps = ps.tile([128, 128], f32, tag="ps_small")
    for k, lt in enumerate((xcT0, xcT1, ones)):
        nc.tensor.matmul(xc1_ps[:], lt[:], fc1Wk[:, k, :], start=(k == 0), stop=(k == 2))
    xc1 = sb.tile([128, 128], f32, tag="xc1")
    nc.scalar.activation(xc1[:], xc1_ps[:],
                         mybir.ActivationFunctionType.Lrelu, alpha=0.01)
    xc1T = sb.tile([128, 128], f32, tag="xc1T")
    t3 = pe_T(xc1[:], 128)
    nc.vector.tensor_copy(xc1T[:], t3[:])

    fc2Wk = sb.tile([128, 2, 32], f32, tag="fc2w")
    nc.sync.dma_start(fc2Wk[:], fc2WkD.ap().rearrange("k r n -> r k n"))
    xc2_ps = ps.tile([128, 32], f32, tag="ps_small")
    for k, lt in enumerate((xc1T, ones)):
        nc.tensor.matmul(xc2_ps[:], lt[:], fc2Wk[:, k, :], start=(k == 0), stop=(k == 1))
    xc2 = sb.tile([128, 32], f32, tag="xc2")
    nc.scalar.activation(xc2[:], xc2_ps[:],
                         mybir.ActivationFunctionType.Lrelu, alpha=0.01)
    xc2T = sb.tile([128, 128], f32, tag="xc2T")
    nc.vector.memset(xc2T[:], 0.0)
    t4 = pe_T(xc2[:], 32)
    nc.vector.tensor_copy(xc2T[0:32, :], t4[:])
    nc.vector.memset(xc2T[32:33, :], 1.0)

    outWk = sb.tile([128, 1], f32, tag="outw")
    nc.sync.dma_start(outWk[:], outWkD[:])
    out_ps = ps.tile([128, 1], f32, tag="ps_small")
    nc.tensor.matmul(out_ps[:], xc2T[:], outWk[:], start=True, stop=True)
    outsb = sb.tile([128, 1], f32, tag="outsb")
    nc.vector.tensor_copy(outsb[:], out_ps[:])
    nc.sync.dma_start(outD[:], outsb[:])


def _build_program(st1, st2):
    nc = bacc.Bacc("TRN2", target_bir_lowering=False, debug=False, num_devices=NC)
    d = nc.dram_tensor
    identD = d("identity", [128, 128], f32, kind="ExternalInput")
    pool_loc = d("pool_loc", [GPC, 4736], f32)
    pool_glob = d("pool_glob", [G, 4736], f32, addr_space="Shared")

    with tile.TileContext(nc) as tc:
        with (
            tc.tile_pool(name="base", bufs=1) as base,
            tc.tile_pool(name="ps", bufs=2, space="PSUM") as ps,
        ):
            lib = nc.gpsimd.load_library(library_config.mlp)
            ident = base.tile([128, 128], f32, tag="ident")
            nc.sync.dma_start(ident[:], identD[:])
            pools = {"ps": ps, "ident": ident, "lib": lib}

            b1 = Branch(nc, pools, "b1_", st1)
            b2 = Branch(nc, pools, "b2_", st2)
            b1.load_residents(base)
            b2.load_residents(base)

            # P1: T builds + GAT b1 aggregation
            with tc.tile_pool(name="p1", bufs=2) as sb:
                stg0 = sb.tile([GPC, 4736], f32, tag="stg0", bufs=1)
                nc.vector.memset(stg0[:], 0.0)
                nc.vector.memset(stg0[:, 4608:4609], 1.0)
                plw = nc.sync.dma_start(pool_loc[:], stg0[:])
                b1.load_jj(sb)
                b1.emit_tbuild(sb)
                b2.emit_tbuild(sb)
                for b in range(st1["NBLK"]):
                    b1.emit_gat_block(sb, b)

            # P2: z-pass(b1) || GAT-agg(b2); AG_z(b1)
            with tc.tile_pool(name="p2", bufs=2) as sb:
                b2.load_jj(sb)
                h1, h2 = _halves(st1["NBLK"])
                mid = st2["NBLK"] // 2
                b1.emit_zpass_half(sb, h1)
                for b in range(0, mid):
                    b2.emit_gat_block(sb, b)
                b1.emit_zpass_half(sb, h2)
                b1.emit_ag_z()
                for b in range(mid, st2["NBLK"]):
                    b2.emit_gat_block(sb, b)

            # P3: z-pass(b2) || GCN-agg(b1); AG_z(b2)
            with tc.tile_pool(name="p3", bufs=2) as sb:
                h1, h2 = _halves(st2["NBLK"])
                mid = st1["NBLK"] // 2
                b2.emit_zpass_half(sb, h1)
                for b in range(0, mid):
                    b1.emit_gcn_block(sb, b)
                b2.emit_zpass_half(sb, h2)
                b2.emit_ag_z()
                for b in range(mid, st1["NBLK"]):
                    b1.emit_gcn_block(sb, b)

            # P4: GCN-W(b1) + pooling(b1) || GCN-agg(b2)
            ws = []
            with tc.tile_pool(name="p4", bufs=2) as sb:
                b1.alloc_zfin(sb)
                h1, h2 = _halves(st1["NBLK"])
                mid = st2["NBLK"] // 2
                b1.emit_gcnw_half(sb, h1)
                for b in range(0, mid):
                    b2.emit_gcn_block(sb, b)
                b1.emit_gcnw_half(sb, h2)
                for b in range(mid, st2["NBLK"]):
                    b2.emit_gcn_block(sb, b)
                ws.extend(b1.emit_pooling(sb, pool_loc, 0, plw))

            # P5: GCN-W(b2) + pooling(b2)
            with tc.tile_pool(name="p5", bufs=2) as sb:
                b2.alloc_zfin(sb)
                for blks in _halves(st2["NBLK"]):
                    b2.emit_gcnw_half(sb, blks)
                ws.extend(b2.emit_pooling(sb, pool_loc, 2304, plw))

            ag_pool = nc.gpsimd.collective_compute(
                "AllGather", mybir.AluOpType.bypass,
                replica_groups=[list(range(NC))],
                ins=[pool_loc[:]], outs=[pool_glob[:]])
            add_dep_helper(ag_pool.ins, plw.ins, reason="AG pool after init")
            for w in ws:
                add_dep_helper(ag_pool.ins, w.ins, reason="AG pool after stages")
            with tc.tile_pool(name="tail", bufs=2) as sb:
                pools["sb"] = sb
                _build_tail(nc, pools, pool_glob, ag_pool)

    nc.compile()
    return nc


# ---------------------------------------------------------------- entry point
def kernel(**inputs) -> np.ndarray:
    st1, pc1, sh1 = _prep_branch(inputs["x1"], inputs["edge_index1"], inputs["batch1"],
                                 inputs["p1_gatW"], inputs["p1_att_src"],
                                 inputs["p1_att_dst"])
    st2, pc2, sh2 = _prep_branch(inputs["x2"], inputs["edge_index2"], inputs["batch2"],
                                 inputs["p2_gatW"], inputs["p2_att_src"],
                                 inputs["p2_att_dst"])
    gatWk1, gcnWk1 = _pack_branch_weights(inputs["p1_gatW"], inputs["p1_gatb"],
                                          inputs["p1_gcnW"], inputs["p1_gcnb"])
    gatWk2, gcnWk2 = _pack_branch_weights(inputs["p2_gatW"], inputs["p2_gatb"],
                                          inputs["p2_gcnW"], inputs["p2_gcnb"])
    tail = _pack_tail(inputs)

    key = (st1["MAXG"], st1["NSUBS"], st1["NGS"], st2["MAXG"], st2["NSUBS"],
           st2["NGS"])
    if key not in _PROG_CACHE:
        _PROG_CACHE[key] = _build_program(st1, st2)
    nc = _PROG_CACHE[key]

    in_maps = []
    for c in range(NC):
        m = {"identity": tail["identity"], "target": tail["target"],
             "fcxtWk": tail["fcxtWk"], "fc1Wk": tail["fc1Wk"],
             "fc2Wk": tail["fc2Wk"], "outWk": tail["outWk"],
             "p1_fcg1Wk": tail["p1_fcg1Wk"], "p1_fcg2Wk": tail["p1_fcg2Wk"],
             "p2_fcg1Wk": tail["p2_fcg1Wk"], "p2_fcg2Wk": tail["p2_fcg2Wk"]}
        for pfx, pc, sh, gatWk, gcnWk in (("b1_", pc1, sh1, gatWk1, gcnWk1),
                                          ("b2_", pc2, sh2, gatWk2, gcnWk2)):
            p = pc[c]
            m[pfx + "xslot"] = sh["xslot"]
            m[pfx + "xslotT"] = sh["xslotT"]
            m[pfx + "Wsd"] = sh["Wsd"]
            m[pfx + "gatWk"] = gatWk
            m[pfx + "gcnWk"] = gcnWk
            for k in ("isrc", "idst", "mask", "cols", "poison", "rcnt"):
                m[pfx + k] = p[k]
        in_maps.append(m)

    res = run_bass_kernel_spmd(nc, in_maps, list(range(NC)))
    global LAST_RES
    LAST_RES = res
    return np.asarray(res.results[0]["out"], dtype=np.float32)


LAST_RES = None


# revision 23
# speedup vs baseline: 1.5268x; 1.0086x over previous
"""Trainium2 Bass kernel for nn_GAT_GCN (GAT conv + GCN conv + pooling + MLP tail).

Strategy (8 NeuronCores, SPMD, full inputs in / full output out), v3:
  - Nodes live in graph-aligned "slots" (MAXG per graph, 16 graphs per core);
    core c owns graphs [16c, 16c+16).  Edges are sharded by destination node
    and bucketed per 128-slot destination block; a host-built one-hot mask
    turns the per-destination segment-sum into TensorEngine matmuls.
  - Per-node tables T = [x | a_src] and T2 = [a_dst] (256 B bf16 rows) are
    built REPLICATED on every core from sequential reads of x (no collective,
    no index gathers).  Per-edge payloads come from dma_gathers of T[src] and
    T2[dst]; gather lengths are trimmed to the per-block max edge count
    (rounded to 16) instead of a global pad.
  - GAT: exp(lrelu(a_s+a_d)) per edge; softmax + symmetric-norm scaling are
    folded into the aggregated y tile (scaled by dinv/den per head on the
    Scalar engine) before the head-blocked weight multiply, which uses
    half-branch-batched DMA-transpose loads.  z := lrelu(dinv * GAT_out).
  - z is AllGathered (bf16) once per branch; the GCN aggregation gathers
    z[src] rows as the mask-matmul rhs; dinv[dst] applied on output.  The
    dense 1152x1152 multiply runs as a transposed pass producing zfin^T,
    which feeds max/mean pooling via free-dim reductions.
  - Emission is interleaved across branches at block granularity so that
    every engine queue always holds independent work:  z-pass(b1) overlaps
    GAT-agg(b2), z-pass(b2) overlaps GCN-agg(b1), GCN-W(b1) overlaps
    GCN-agg(b2), and each branch's z-AllGather overlaps the other branch's
    compute.  The tiny MLP tail is replicated on every core.

Host-side preprocessing is restricted to index manipulation (sorting/
bucketing edges, one-hot masks, padding, row permutations) and parameter
repacking (padding / bf16 casts / tiny reshapes) -- all data-dependent float
compute runs on device.
"""

import numpy as np
import ml_dtypes

import concourse.bacc as bacc
import concourse.tile as tile
from concourse import mybir, library_config
from concourse.bass_utils import run_bass_kernel_spmd
from concourse.tile_rust import add_dep_helper

# ---------------------------------------------------------------- constants
N = 20000
E = 160000
G = 128
F = 114
H = 10
HF = 1140          # F * H
NC = 8
GPC = G // NC      # graphs per core
P = 128

bf16 = mybir.dt.bfloat16
f32 = mybir.dt.float32
i16 = mybir.dt.int16

BF = ml_dtypes.bfloat16

_PROG_CACHE: dict = {}


# ---------------------------------------------------------------- host utils
def _wrap_idx(idx: np.ndarray) -> np.ndarray:
    """int16 index list (len % 16 == 0) -> [128, len/16] wrapped layout."""
    n = idx.shape[0]
    assert n % 16 == 0
    return np.tile(idx.reshape(-1, 16).T, (8, 1)).astype(np.int16)


def _prep_branch(x, ei, batch, gatW, att_src, att_dst):
    """Host preprocessing for one branch. Returns (static, percore, shared)."""
    x = np.asarray(x, dtype=np.float32)
    ei = np.asarray(ei).astype(np.int64)
    batch = np.asarray(batch).astype(np.int64)
    gatW = np.asarray(gatW, dtype=np.float32)
    att_src = np.asarray(att_src, dtype=np.float32)
    att_dst = np.asarray(att_dst, dtype=np.float32)

    cnt = np.bincount(batch, minlength=G)
    MAXG = int(cnt.max())
    SS = ((16 * MAXG + 127) // 128) * 128       # slots per core shard
    NBLK = SS // 128
    assert NC * SS <= 32768, "row ids must fit int16"

    gstart = np.zeros(G + 1, np.int64)
    np.cumsum(cnt, out=gstart[1:])
    nodes = np.arange(N)
    rank = nodes - gstart[batch]
    slot_of_node = (batch % GPC) * MAXG + rank          # [N] in [0, 16*MAXG)
    core_of_node = batch // GPC                          # [N]
    row_of_node = core_of_node * SS + slot_of_node       # [N] global table row

    src = np.concatenate([ei[0], nodes])
    dst = np.concatenate([ei[1], nodes])

    core_e = core_of_node[dst]
    slot_e = slot_of_node[dst]
    blk_e = slot_e // 128
    dloc_e = slot_e % 128

    # bucket edges per (core, block)
    order = np.lexsort((blk_e, core_e))
    src_s = src[order]
    dst_s = dst[order]
    blk_s, dloc_s = blk_e[order], dloc_e[order]
    key = core_e[order] * NBLK + blk_s
    counts = np.bincount(key, minlength=NC * NBLK)
    starts = np.zeros(NC * NBLK + 1, np.int64)
    np.cumsum(counts, out=starts[1:])
    co = counts.reshape(NC, NBLK)
    # exact subtile / gather-length per block, shared across cores
    maxcnt = co.max(axis=0)
    NSUBS = tuple(int(v) for v in np.maximum(1, -(-maxcnt // 128)))
    NGS = tuple(int(v) for v in np.maximum(16, 16 * (-(-maxcnt // 16))))
    TOTSUB = sum(NSUBS)
    TOTNG = sum(NGS)
    NSUBMAX = max(NSUBS)

    deg = np.bincount(dst, minlength=N).astype(np.float64)
    dinv_node = 1.0 / np.sqrt(np.maximum(deg, 1.0))

    slots = np.arange(SS)
    g_local = np.minimum(slots // MAXG, GPC - 1)
    r_local = slots - g_local * MAXG

    ngoff = np.zeros(NBLK + 1, np.int64)
    np.cumsum(NGS, out=ngoff[1:])

    percore = []
    for c in range(NC):
        isrc = np.zeros(TOTNG, np.int64)
        mask = np.zeros((128, TOTSUB, 128), np.float32)   # [e, (b s), d]
        soff = 0
        for b in range(NBLK):
            k = c * NBLK + b
            cnt_b = int(counts[k])
            sl = slice(starts[k], starts[k + 1])
            o = ngoff[b]
            isrc[o:o + cnt_b] = row_of_node[src_s[sl]]
            pos = np.arange(cnt_b)
            mask[pos % 128, soff + pos // 128, dloc_s[sl]] = 1.0
            soff += NSUBS[b]

        isrc_w = np.concatenate(
            [_wrap_idx(isrc[ngoff[b]:ngoff[b + 1]].astype(np.int16))
             for b in range(NBLK)], axis=1)

        g_global = c * GPC + g_local
        validity = (slots < 16 * MAXG) & (r_local < cnt[g_global])
        node_of_slot = np.zeros(SS, np.int64)
        real = validity.nonzero()[0]
        node_of_slot[real] = gstart[g_global[real]] + r_local[real]

        dinv_slot = np.ones(SS, np.float32)
        dinv_slot[real] = dinv_node[node_of_slot[real]]
        valid = validity.astype(np.float32)

        cols = np.zeros((128, NBLK, 4), np.float32)
        cols[:, :, 0] = dinv_slot.reshape(NBLK, 128).T
        cols[:, :, 1] = (1.0 - valid).reshape(NBLK, 128).T
        cols[:, :, 2] = valid.reshape(NBLK, 128).T

        poison = np.where(validity, 0.0, -1e28).astype(np.float32)
        rcnt = (1.0 / np.maximum(cnt[c * GPC:(c + 1) * GPC], 1.0)).astype(np.float32)

        percore.append({
            "isrc": isrc_w,                                          # [128, TOTNG/16]
            "mask": mask.astype(BF),                                 # [128, TOTSUB, 128]
            "cols": cols,                                            # [128, NBLK, 4]
            "poison": np.broadcast_to(poison.astype(BF), (128, SS)).copy(),
            "rcnt": np.broadcast_to(rcnt, (128, GPC)).copy(),
        })

    # shared tables: slot-ordered x (replicated T build), plus its transpose
    xslot = np.zeros((NC * SS, 128), np.float32)
    for c in range(NC):
        g_global = c * GPC + g_local
        validity = (slots < 16 * MAXG) & (r_local < cnt[g_global])
        real = validity.nonzero()[0]
        node = gstart[g_global[real]] + r_local[real]
        xslot[c * SS + real, :F] = x[node]
    xslot_bf = xslot.astype(BF)
    for c in range(NC):
        percore[c]["xownT"] = np.ascontiguousarray(
            xslot_bf[c * SS:(c + 1) * SS].T)                # [128, SS]

    gatW3 = gatW.reshape(F, H, F)
    Ws = np.einsum("khf,hf->kh", gatW3, att_src)
    Wd = np.einsum("khf,hf->kh", gatW3, att_dst)
    Wsd = np.zeros((128, 2 * H), np.float32)
    Wsd[:F, :H] = Ws
    Wsd[:F, H:] = Wd

    shared = {
        "xslot": xslot_bf,                                  # [NC*SS, 128]
        "xslotT": np.ascontiguousarray(xslot_bf.T),         # [128, NC*SS]
        "Wsd": Wsd.astype(BF),                              # [128, 20]
    }
    static = dict(MAXG=MAXG, SS=SS, NBLK=NBLK, NSUBS=NSUBS, NGS=NGS,
                  TOTSUB=TOTSUB, TOTNG=TOTNG, NSUBMAX=NSUBMAX)
    return static, percore, shared


def _pack_branch_weights(gatW, gatb, gcnW, gcnb):
    gatW = np.asarray(gatW, np.float32)
    gatb = np.asarray(gatb, np.float32)
    gcnW = np.asarray(gcnW, np.float32)
    gcnb = np.asarray(gcnb, np.float32)
    gatWk = np.zeros((H, 128, F), np.float32)
    gatW3 = gatW.reshape(F, H, F)
    for h in range(H):
        gatWk[h, :F, :] = gatW3[:, h, :]
        gatWk[h, F, :] = gatb[h * F:(h + 1) * F]
    gcn_pad = np.zeros((1152, 1152), np.float32)
    gcn_pad[:HF, :HF] = gcnW
    gcn_pad[HF, :HF] = gcnb
    gcnWk = gcn_pad.reshape(9, 128, 1152)
    return gatWk.astype(BF), gcnWk.astype(BF)


def _pack_tail(inp):
    f = lambda k: np.asarray(inp[k], np.float32)
    tail = {}
    for p in ("p1", "p2"):
        W1 = np.zeros((2432, 1024), np.float32)
        fg1 = f(p + "_fcg1W")           # [2280, 1000]
        W1[0:HF, 0:1000] = fg1[0:HF]
        W1[1152:1152 + HF, 0:1000] = fg1[HF:2 * HF]
        k1 = np.concatenate([W1[:2304].reshape(18, 128, 1024),
                             np.zeros((1, 128, 1024), np.float32)], axis=0)
        k1[18, 0, 0:1000] = f(p + "_fcg1b")
        tail[p + "_fcg1Wk"] = k1.astype(np.float32)
        W2 = np.zeros((1024, 64), np.float32)
        W2[0:1000] = f(p + "_fcg2W")
        W2[1000] = f(p + "_fcg2b")
        tail[p + "_fcg2Wk"] = W2.reshape(8, 128, 64).astype(np.float32)
    Wx = np.zeros((1024, 128), np.float32)
    Wx[0:1000] = f("fcxtW")
    Wx[1000] = f("fcxtb")
    tail["fcxtWk"] = Wx.reshape(8, 128, 128).astype(np.float32)
    W1 = np.zeros((3, 128, 128), np.float32)
    W1[0] = f("fc1W")[0:128]
    W1[1] = f("fc1W")[128:256]
    W1[2, 0] = f("fc1b")
    tail["fc1Wk"] = W1.astype(np.float32)
    W2 = np.zeros((2, 128, 32), np.float32)
    W2[0] = f("fc2W")
    W2[1, 0] = f("fc2b")
    tail["fc2Wk"] = W2.astype(np.float32)
    Wo = np.zeros((128, 1), np.float32)
    Wo[0:32, 0] = f("outW")[:, 0]
    Wo[32, 0] = float(np.asarray(inp["outb"]).reshape(-1)[0])
    tail["outWk"] = Wo.astype(np.float32)
    tail["identity"] = np.eye(128, dtype=np.float32)
    tail["target"] = f("target")
    return tail


# ---------------------------------------------------------------- device build
GATHER_MAX = 1024  # dma_gather breaks above 1024 indices per call


class Branch:
    """Per-branch emission state: DRAM handles, resident tiles, generators."""

    def __init__(self, nc, pools, pfx, st):
        self.nc = nc
        self.pools = pools
        self.pfx = pfx
        self.st = st
        MAXG, SS, NBLK = st["MAXG"], st["SS"], st["NBLK"]
        TOTSUB, TOTNG = st["TOTSUB"], st["TOTNG"]
        d = nc.dram_tensor
        self.xslotD = d(pfx + "xslot", [NC * SS, 128], bf16, kind="ExternalInput")
        self.xslotTD = d(pfx + "xslotT", [128, NC * SS], bf16, kind="ExternalInput")
        self.xownTD = d(pfx + "xownT", [128, SS], bf16, kind="ExternalInput")
        self.WsdD = d(pfx + "Wsd", [128, 2 * H], bf16, kind="ExternalInput")
        self.isrcD = d(pfx + "isrc", [128, TOTNG // 16], i16, kind="ExternalInput")
        self.maskD = d(pfx + "mask", [128, TOTSUB, 128], bf16, kind="ExternalInput")
        self.colsD = d(pfx + "cols", [128, NBLK, 4], f32, kind="ExternalInput")
        self.gatWkD = d(pfx + "gatWk", [H, 128, F], bf16, kind="ExternalInput")
        self.gcnWkD = d(pfx + "gcnWk", [9, 128, 1152], bf16, kind="ExternalInput")
        self.poisonD = d(pfx + "poison", [128, SS], bf16, kind="ExternalInput")
        self.rcntD = d(pfx + "rcnt", [128, GPC], f32, kind="ExternalInput")
        self.TD = d(pfx + "T", [NC * SS, 128], bf16)
        self.y_dram = d(pfx + "y", [SS, 1280], bf16)
        self.z_loc = d(pfx + "z_loc", [SS, 1152], bf16)
        self.z_glob = d(pfx + "z_glob", [NC * SS, 1152], bf16, addr_space="Shared")
        self.y2_dram = d(pfx + "y2", [SS, 1152], bf16)

        self.fence = None
        self.y_writes = []
        self.z_writes = []
        self.y2_writes = []
        self.ag_z = None

        # offsets
        self.soffs = np.zeros(NBLK + 1, np.int64)
        np.cumsum(st["NSUBS"], out=self.soffs[1:])
        self.ioffs = np.zeros(NBLK + 1, np.int64)
        np.cumsum([n // 16 for n in st["NGS"]], out=self.ioffs[1:])

    # ---- resident tiles (call once inside the long-lived base pool) ----
    def load_residents(self, base):
        nc = self.nc
        self.gatWk = base.tile([128, H, F], bf16, tag=self.pfx + "gatwk")
        nc.sync.dma_start(self.gatWk[:], self.gatWkD.ap().rearrange("h k n -> k h n"))
        self.cols = base.tile([128, self.st["NBLK"], 4], f32, tag=self.pfx + "cols")
        nc.sync.dma_start(self.cols[:], self.colsD[:])
        self.ii = base.tile([128, self.st["TOTNG"] // 16], i16, tag=self.pfx + "ii")
        nc.sync.dma_start(self.ii[:], self.isrcD[:])
        self.Wsd = base.tile([128, 2 * H], bf16, tag=self.pfx + "wsd")
        nc.sync.dma_start(self.Wsd[:], self.WsdD[:])
        self.xownT = base.tile([128, self.st["SS"]], bf16, tag=self.pfx + "xownT")
        nc.sync.dma_start(self.xownT[:], self.xownTD[:])

    def _gather(self, out_tile, table_ap, idx_tile, b, elem, deps):
        nc, pools = self.nc, self.pools
        ng = self.st["NGS"][b]
        i0 = int(self.ioffs[b])
        if ng % 128:
            # pre-zero the partial last subtile; the gather then overwrites
            # rows [0, ng%128) of it (DVE memsets need aligned partition 0)
            nc.vector.memset(out_tile[:, ng // 128, :], 0.0)
        insts = []
        for o in range(0, ng, GATHER_MAX):
            n = min(ng, o + GATHER_MAX) - o
            g = nc.gpsimd.dma_gather(
                out_tile[:, o // 128:(o + n + 127) // 128, :], table_ap,
                idx_tile[:, i0 + o // 16:i0 + (o + n) // 16], n, n, elem)
            add_dep_helper(g.ins, pools["lib"].ins, reason="gather after lib")
            for dd in deps:
                add_dep_helper(g.ins, dd.ins, reason="gather dep")
            insts.append(g)
        return insts

    # ---- T build ----
    def emit_tbuild(self, sb):
        nc, ps = self.nc, self.pools["ps"]
        writes = []
        TOT = NC * self.st["SS"]
        for r0 in range(0, TOT, 2048):
            rows = min(2048, TOT - r0)
            na = rows // 128
            xt = sb.tile([128, 2048], bf16, tag="xt")
            nc.sync.dma_start(xt[:, 0:rows], self.xslotTD[:, r0:r0 + rows])
            xr = sb.tile([128, 16, 128], bf16, tag="xr")
            nc.sync.dma_start(
                xr[:, 0:na, :],
                self.xslotD[r0:r0 + rows, :].rearrange("(a p) c -> p a c", p=128))
            aps = ps.tile([128, 16, H], f32, tag="ps_small")
            for a in range(na):
                nc.tensor.matmul(aps[:, a, :], xt[:, a * 128:(a + 1) * 128],
                                 self.Wsd[:, 0:H], start=True, stop=True)
            Tt = sb.tile([128, 16, 128], bf16, tag="Tt")
            nc.vector.tensor_copy(Tt[:, 0:na, 0:F], xr[:, 0:na, 0:F])
            nc.vector.tensor_copy(Tt[:, 0:na, F:F + H], aps[:, 0:na, :])
            nc.vector.memset(Tt[:, 0:na, F + H:128], 0.0)
            w1 = nc.sync.dma_start(
                self.TD[r0:r0 + rows, :].rearrange("(a p) c -> p a c", p=128),
                Tt[:, 0:na, :])
            writes.append(w1)
        fence_t = sb.tile([128, 1], f32, tag="fence" + self.pfx)
        self.fence = nc.vector.memset(fence_t[:], 0.0)
        for w in writes:
            add_dep_helper(self.fence.ins, w.ins, reason="T fence")

    # ---- GAT aggregation, one block ----
    def prezero_gat(self, sb):
        nc = self.nc
        NSM = self.st["NSUBMAX"]
        for _ in range(2):
            S = sb.tile([128, NSM, 128], bf16, tag="S")
            nc.vector.memset(S[:], 0.0)
            D = sb.tile([128, NSM, 128], bf16, tag="D")
            nc.vector.memset(D[:], 0.0)

    def emit_gat_block(self, sb, b):
        nc, ps = self.nc, self.pools["ps"]
        ns = self.st["NSUBS"][b]
        NSM = self.st["NSUBMAX"]
        soff = int(self.soffs[b])
        cols = self.cols

        S = sb.tile([128, NSM, 128], bf16, tag="S")
        self._gather(S, self.TD[:], self.ii, b, 128, (self.fence,))
        Mt = sb.tile([128, NSM, 128], bf16, tag="Mt")
        nc.sync.dma_start(Mt[:, 0:ns, :], self.maskD[:, soff:soff + ns, :])

        # a_d of this block's 128 destination slots, from local x^T
        adb_ps = ps.tile([128, H], f32, tag="ps_small")
        nc.tensor.matmul(adb_ps[:], self.xownT[:, b * 128:(b + 1) * 128],
                         self.Wsd[:, H:2 * H], start=True, stop=True)
        adb = sb.tile([128, H], bf16, tag="adb")
        nc.scalar.copy(adb[:], adb_ps[:])

        # broadcast a_d[dst] to edge rows via transposed-mask matmuls
        lg = sb.tile([128, NSM, H], f32, tag="lg")
        identb = self.pools["identb"]
        for s in range(ns):
            mtt_ps = ps.tile([128, 128], bf16, tag="ps_small")
            nc.tensor.transpose(mtt_ps[:], Mt[:, s, :], identb[:])
            mtt = sb.tile([128, 128], bf16, tag="mtt")
            nc.scalar.copy(mtt[:], mtt_ps[:])
            ad_ps = ps.tile([128, H], f32, tag="ps_small")
            nc.tensor.matmul(ad_ps[:], mtt[:], adb[:], start=True, stop=True)
            nc.vector.tensor_tensor(out=lg[:, s, :], in0=S[:, s, F:F + H],
                                    in1=ad_ps[:], op=mybir.AluOpType.add)
        l3 = sb.tile([128, NSM, H], f32, tag="l3")
        nc.vector.scalar_tensor_tensor(out=l3[:, 0:ns, :], in0=lg[:, 0:ns, :],
                                       scalar=0.2, in1=lg[:, 0:ns, :],
                                       op0=mybir.AluOpType.mult,
                                       op1=mybir.AluOpType.max)
        exb = sb.tile([128, NSM, H], bf16, tag="exb")
        nc.scalar.activation(exb[:, 0:ns, :], l3[:, 0:ns, :],
                             mybir.ActivationFunctionType.Exp)

        R = sb.tile([128, NSM, 1152], bf16, tag="R")
        nc.vector.tensor_tensor(
            out=R[:, 0:ns, 0:HF].rearrange("p s (h f) -> p s h f", h=H),
            in0=S[:, 0:ns, 0:F].unsqueeze(2).broadcast_to([128, ns, H, F]),
            in1=exb[:, 0:ns, :].unsqueeze(3).broadcast_to([128, ns, H, F]),
            op=mybir.AluOpType.mult)
        nc.scalar.copy(R[:, 0:ns, HF:1150], exb[:, 0:ns, :])

        y_ps = ps.tile([128, 1280], f32, tag="ps_big")
        for s in range(ns):
            for c0, c1 in ((0, 512), (512, 1024), (1024, 1150)):
                nc.tensor.matmul(y_ps[:, c0:c1], Mt[:, s, :], R[:, s, c0:c1],
                                 start=(s == 0), stop=(s == ns - 1))

        den = sb.tile([128, H], f32, tag="den")
        nc.vector.tensor_scalar(out=den[:], in0=y_ps[:, HF:1150],
                                scalar1=cols[:, b, 1:2], scalar2=None,
                                op0=mybir.AluOpType.add)
        rden = sb.tile([128, H], f32, tag="rden")
        nc.vector.reciprocal(rden[:], den[:])
        rdn = sb.tile([128, H], f32, tag="rdn")
        nc.vector.tensor_scalar(out=rdn[:], in0=rden[:], scalar1=cols[:, b, 0:1],
                                scalar2=None, op0=mybir.AluOpType.mult)

        y_sb = sb.tile([128, 1280], bf16, tag="ysb")
        ytv = y_sb[:].rearrange("p (h c) -> p h c", h=H)
        ypv = y_ps[:, 0:HF].rearrange("p (h f) -> p h f", h=H)
        for h in range(H):
            nc.scalar.activation(ytv[:, h, 0:F], ypv[:, h, :],
                                 mybir.ActivationFunctionType.Copy,
                                 scale=rdn[:, h:h + 1])
        nc.vector.tensor_copy(
            ytv[:, :, F:F + 1],
            cols[:, b, 0:1].unsqueeze(1).broadcast_to([128, H, 1]))
        nc.vector.memset(ytv[:, :, F + 1:128], 0.0)
        w = nc.sync.dma_start(self.y_dram[b * 128:(b + 1) * 128, :], y_sb[:])
        self.y_writes.append(w)

    # ---- z production: one half (list of blocks), transposes batched ----
    def emit_zpass_half(self, sb, blocks):
        nc, ps = self.nc, self.pools["ps"]
        t0, t1 = blocks[0], blocks[-1] + 1
        rows = (t1 - t0) * 128
        yTs = []
        for h in range(H):
            yT = sb.tile([128, 13 * 128], bf16, tag=f"yT{h}", bufs=1)
            ld = nc.sync.dma_start_transpose(
                out=yT[:, 0:rows],
                in_=self.y_dram[t0 * 128:t1 * 128, h * 128:(h + 1) * 128])
            for t in blocks:
                add_dep_helper(ld.ins, self.y_writes[t].ins, reason="yT after y")
            yTs.append(yT)
        for t in blocks:
            o = (t - t0) * 128
            zp = ps.tile([128, 1280], f32, tag="ps_big")
            for h in range(H):
                nc.tensor.matmul(zp[:, h * 128:h * 128 + F],
                                 yTs[h][:, o:o + 128], self.gatWk[:, h, :],
                                 start=(h % 4 == 0), stop=(h % 4 == 3) or (h == H - 1))
            zt = sb.tile([128, 1152], bf16, tag="zt")
            nc.scalar.activation(
                zt[:, 0:HF].rearrange("p (h f) -> p h f", h=H),
                zp[:].rearrange("p (h c) -> p h c", h=H)[:, :, 0:F],
                mybir.ActivationFunctionType.Lrelu, alpha=0.01)
            nc.vector.memset(zt[:, HF:1152], 0.0)
            w = nc.sync.dma_start(self.z_loc[t * 128:(t + 1) * 128, :], zt[:])
            self.z_writes.append(w)

    def emit_ag_z(self):
        nc = self.nc
        self.ag_z = nc.gpsimd.collective_compute(
            "AllGather", mybir.AluOpType.bypass,
            replica_groups=[list(range(NC))],
            ins=[self.z_loc[:]], outs=[self.z_glob[:]])
        for w in self.z_writes:
            add_dep_helper(self.ag_z.ins, w.ins, reason="AG_z after z writes")

    # ---- GCN aggregation, one block ----
    def prezero_gcn(self, sb):
        nc = self.nc
        NSM = self.st["NSUBMAX"]
        for _ in range(2):
            Z = sb.tile([128, NSM, 1152], bf16, tag="Z")
            nc.vector.memset(Z[:], 0.0)

    def emit_gcn_block(self, sb, b):
        nc, ps = self.nc, self.pools["ps"]
        ns = self.st["NSUBS"][b]
        NSM = self.st["NSUBMAX"]
        soff = int(self.soffs[b])
        cols = self.cols

        Z = sb.tile([128, NSM, 1152], bf16, tag="Z")
        self._gather(Z, self.z_glob[:], self.ii, b, 1152, (self.ag_z,))
        Mt = sb.tile([128, NSM, 128], bf16, tag="Mt2")
        nc.sync.dma_start(Mt[:, 0:ns, :], self.maskD[:, soff:soff + ns, :])

        y2_ps = ps.tile([128, 1280], f32, tag="ps_big")
        for s in range(ns):
            for c0, c1 in ((0, 512), (512, 1024), (1024, HF)):
                nc.tensor.matmul(y2_ps[:, c0:c1], Mt[:, s, :], Z[:, s, c0:c1],
                                 start=(s == 0), stop=(s == ns - 1))
        y2t = sb.tile([128, 1152], bf16, tag="y2t")
        nc.scalar.activation(y2t[:, 0:512], y2_ps[:, 0:512],
                             mybir.ActivationFunctionType.Copy,
                             scale=cols[:, b, 0:1])
        nc.scalar.activation(y2t[:, 512:HF], y2_ps[:, 512:HF],
                             mybir.ActivationFunctionType.Copy,
                             scale=cols[:, b, 0:1])
        nc.vector.tensor_copy(y2t[:, HF:HF + 1], cols[:, b, 2:3])
        nc.vector.memset(y2t[:, HF + 1:1152], 0.0)
        w = nc.sync.dma_start(self.y2_dram[b * 128:(b + 1) * 128, :], y2t[:])
        self.y2_writes.append(w)

    # ---- GCN W-pass: one half (transposes batched), writes zfin slices ----
    def alloc_zfin(self, sb):
        nc = self.nc
        self.gcnWk = sb.tile([128, 9, 1152], bf16, tag=self.pfx + "gcnwk", bufs=1)
        nc.sync.dma_start(self.gcnWk[:],
                          self.gcnWkD.ap().rearrange("kb kr n -> kr kb n"))
        self.zfin = sb.tile([128, 9, self.st["SS"]], bf16,
                            tag=self.pfx + "zfin", bufs=1)

    def emit_gcnw_half(self, sb, blocks):
        nc, ps = self.nc, self.pools["ps"]
        t0, t1 = blocks[0], blocks[-1] + 1
        rows = (t1 - t0) * 128
        r0 = t0 * 128
        yTs = []
        for kb in range(9):
            y2T = sb.tile([128, 13 * 128], bf16, tag=f"y2T{kb}", bufs=1)
            ld = nc.sync.dma_start_transpose(
                out=y2T[:, 0:rows],
                in_=self.y2_dram[r0:r0 + rows, kb * 128:(kb + 1) * 128])
            for t in blocks:
                add_dep_helper(ld.ins, self.y2_writes[t].ins, reason="y2T dep")
            yTs.append(y2T)
        for c0 in range(0, rows, 512):
            cw = min(512, rows - c0)
            for nb in range(9):
                ct = ps.tile([128, 512], f32, tag="ps_small")
                for kb in range(9):
                    nc.tensor.matmul(ct[:, 0:cw],
                                     self.gcnWk[:, kb, nb * 128:(nb + 1) * 128],
                                     yTs[kb][:, c0:c0 + cw],
                                     start=(kb == 0), stop=(kb == 8))
                nc.scalar.activation(self.zfin[:, nb, r0 + c0:r0 + c0 + cw],
                                     ct[:, 0:cw],
                                     mybir.ActivationFunctionType.Lrelu, alpha=0.01)

    # ---- pooling + staging ----
    def emit_pooling(self, sb, pool_loc, pool_col0, plw):
        nc, ps = self.nc, self.pools["ps"]
        MAXG = self.st["MAXG"]
        ident = self.pools["ident"]
        poison = sb.tile([128, self.st["SS"]], bf16, tag=self.pfx + "poison", bufs=1)
        nc.sync.dma_start(poison[:], self.poisonD[:])
        rcnt = sb.tile([128, GPC], f32, tag=self.pfx + "rcnt")
        nc.sync.dma_start(rcnt[:], self.rcntD[:])
        mxT = sb.tile([128, 9, GPC], f32, tag=self.pfx + "mxT")
        smT = sb.tile([128, 9, GPC], f32, tag=self.pfx + "smT")
        for g in range(GPC):
            s0 = g * MAXG
            tmp = sb.tile([128, 9, MAXG], bf16, tag="ptmp")
            nc.vector.tensor_tensor(
                out=tmp[:], in0=self.zfin[:, :, s0:s0 + MAXG],
                in1=poison[:, s0:s0 + MAXG].unsqueeze(1).broadcast_to([128, 9, MAXG]),
                op=mybir.AluOpType.add)
            nc.vector.reduce_max(mxT[:, :, g:g + 1], tmp[:],
                                 axis=mybir.AxisListType.X)
            nc.vector.reduce_sum(smT[:, :, g:g + 1], self.zfin[:, :, s0:s0 + MAXG],
                                 axis=mybir.AxisListType.X)
        mnT = sb.tile([128, 9, GPC], f32, tag=self.pfx + "mnT")
        nc.vector.tensor_tensor(out=mnT[:], in0=smT[:],
                                in1=rcnt[:].unsqueeze(1).broadcast_to([128, 9, GPC]),
                                op=mybir.AluOpType.mult)
        writes = []
        for which, statT in ((0, mxT), (1, mnT)):
            for ft in range(9):
                tp = ps.tile([GPC, 128], f32, tag="ps_small")
                nc.tensor.transpose(tp[:], statT[:, ft, :], ident[:])
                stg = sb.tile([GPC, 128], f32, tag="stg")
                nc.vector.tensor_copy(stg[:], tp[:])
                w = nc.sync.dma_start(
                    pool_loc[:, pool_col0 + which * 1152 + ft * 128:
                             pool_col0 + which * 1152 + ft * 128 + 128], stg[:])
                add_dep_helper(w.ins, plw.ins, reason="stage after pool init")
                writes.append(w)
        return writes


def _halves(NBLK):
    h = (NBLK + 1) // 2
    return [list(range(0, h)), list(range(h, NBLK))]


def _build_tail(nc, pools, pool_glob, ag_pool):
    d = nc.dram_tensor
    sb, ps = pools["sb"], pools["ps"]
    ident = pools["ident"]
    tgtD = d("target", [G, 1000], f32, kind="ExternalInput")
    fcxtWkD = d("fcxtWk", [8, 128, 128], f32, kind="ExternalInput")
    fc1WkD = d("fc1Wk", [3, 128, 128], f32, kind="ExternalInput")
    fc2WkD = d("fc2Wk", [2, 128, 32], f32, kind="ExternalInput")
    outWkD = d("outWk", [128, 1], f32, kind="ExternalInput")
    outD = d("out", [G, 1], f32, kind="ExternalOutput")

    def pe_T(src_ap, rows):
        tp = ps.tile([rows, 128], f32, tag="ps_small")
        nc.tensor.transpose(tp[:], src_ap, ident[:])
        return tp

    def mm_transposed(src_tile, nk, rhs_fn, psum, chunks, tag):
        for k in range(nk):
            tp = pe_T(src_tile[:, k * 128:(k + 1) * 128], 128)
            tt = sb.tile([128, 128], f32, tag=tag)
            nc.vector.tensor_copy(tt[:], tp[:])
            for c0, c1 in chunks:
                nc.tensor.matmul(psum[:, c0:c1], tt[:], rhs_fn(k)[:, c0:c1],
                                 start=(k == 0), stop=(k == nk - 1))

    tg = sb.tile([128, 1024], f32, tag="tg")
    nc.sync.dma_start(tg[:, 0:1000], tgtD[:])
    nc.vector.memset(tg[:, 1000:1001], 1.0)
    nc.vector.memset(tg[:, 1001:1024], 0.0)
    fcxtWk = sb.tile([128, 8, 128], f32, tag="tw8")
    nc.sync.dma_start(fcxtWk[:], fcxtWkD.ap().rearrange("k r n -> r k n"))
    xt_ps = ps.tile([128, 128], f32, tag="ps_small")
    mm_transposed(tg, 8, lambda k: fcxtWk[:, k, :], xt_ps, ((0, 128),), "ttl")
    xt_sb = sb.tile([128, 128], f32, tag="xt2")
    nc.vector.tensor_copy(xt_sb[:], xt_ps[:])

    gvecs = []
    for bi, p in enumerate(("p1", "p2")):
        fg1D = d(p + "_fcg1Wk", [19, 128, 1024], f32, kind="ExternalInput")
        fg2D = d(p + "_fcg2Wk", [8, 128, 64], f32, kind="ExternalInput")
        fg1 = sb.tile([128, 19, 1024], f32, tag="fg1", bufs=1)
        nc.sync.dma_start(fg1[:], fg1D.ap().rearrange("k r n -> r k n"))
        g_ps = ps.tile([128, 1024], f32, tag="ps_big")
        kts = list(range(bi * 18, bi * 18 + 18)) + [36]
        for k, kt in enumerate(kts):
            pl0 = sb.tile([128, 128], f32, tag="pl0")
            ld = nc.sync.dma_start(pl0[:], pool_glob[:, kt * 128:(kt + 1) * 128])
            add_dep_helper(ld.ins, ag_pool.ins, reason="pool load after AG")
            tp = pe_T(pl0[:], 128)
            pl = sb.tile([128, 128], f32, tag="plt")
            nc.vector.tensor_copy(pl[:], tp[:])
            for c0, c1 in ((0, 512), (512, 1024)):
                nc.tensor.matmul(g_ps[:, c0:c1], pl[:], fg1[:, k, c0:c1],
                                 start=(k == 0), stop=(k == 18))
        glr = sb.tile([128, 1024], f32, tag="glr")
        nc.scalar.activation(glr[:, 0:1000], g_ps[:, 0:1000],
                             mybir.ActivationFunctionType.Lrelu, alpha=0.01)
        nc.vector.memset(glr[:, 1000:1001], 1.0)
        nc.vector.memset(glr[:, 1001:1024], 0.0)
        fg2 = sb.tile([128, 8, 64], f32, tag="tw8b")
        nc.sync.dma_start(fg2[:], fg2D.ap().rearrange("k r n -> r k n"))
        g2_ps = ps.tile([128, 64], f32, tag="ps_small")
        mm_transposed(glr, 8, lambda k: fg2[:, k, :], g2_ps, ((0, 64),), "gtl")
        gv = sb.tile([128, 64], f32, tag=f"gv{bi}")
        nc.vector.tensor_copy(gv[:], g2_ps[:])
        gvecs.append(gv)

    xcT0 = sb.tile([128, 128], f32, tag="xcT0")
    t0 = pe_T(gvecs[0][:], 64)
    nc.vector.tensor_copy(xcT0[0:64, :], t0[:])
    t1 = pe_T(gvecs[1][:], 64)
    nc.vector.tensor_copy(xcT0[64:128, :], t1[:])
    xcT1 = sb.tile([128, 128], f32, tag="xcT1")
    t2 = pe_T(xt_sb[:], 128)
    nc.vector.tensor_copy(xcT1[:], t2[:])
    ones = sb.tile([128, 128], f32, tag="ones")
    nc.vector.memset(ones[:], 0.0)
    nc.vector.memset(ones[0:1, :], 1.0)

    fc1Wk = sb.tile([128, 3, 128], f32, tag="fc1w")
    nc.sync.dma_start(fc1Wk[:], fc1WkD.ap().rearrange("k r n -> r k n"))
    xc1_ps = ps.tile([128, 128], f32, tag="ps_small")
    for k, lt in enumerate((xcT0, xcT1, ones)):
        nc.tensor.matmul(xc1_ps[:], lt[:], fc1Wk[:, k, :], start=(k == 0), stop=(k == 2))
    xc1 = sb.tile([128, 128], f32, tag="xc1")
    nc.scalar.activation(xc1[:], xc1_ps[:],
                         mybir.ActivationFunctionType.Lrelu, alpha=0.01)
    xc1T = sb.tile([128, 128], f32, tag="xc1T")
    t3 = pe_T(xc1[:], 128)
    nc.vector.tensor_copy(xc1T[:], t3[:])

    fc2Wk = sb.tile([128, 2, 32], f32, tag="fc2w")
    nc.sync.dma_start(fc2Wk[:], fc2WkD.ap().rearrange("k r n -> r k n"))
    xc2_ps = ps.tile([128, 32], f32, tag="ps_small")
    for k, lt in enumerate((xc1T, ones)):
        nc.tensor.matmul(xc2_ps[:], lt[:], fc2Wk[:, k, :], start=(k == 0), stop=(k == 1))
    xc2 = sb.tile([128, 32], f32, tag="xc2")
    nc.scalar.activation(xc2[:], xc2_ps[:],
                         mybir.ActivationFunctionType.Lrelu, alpha=0.01)
    xc2T = sb.tile([128, 128], f32, tag="xc2T")
    nc.vector.memset(xc2T[:], 0.0)
    t4 = pe_T(xc2[:], 32)
    nc.vector.tensor_copy(xc2T[0:32, :], t4[:])
    nc.vector.memset(xc2T[32:33, :], 1.0)

    outWk = sb.tile([128, 1], f32, tag="outw")
    nc.sync.dma_start(outWk[:], outWkD[:])
    out_ps = ps.tile([128, 1], f32, tag="ps_small")
    nc.tensor.matmul(out_ps[:], xc2T[:], outWk[:], start=True, stop=True)
    outsb = sb.tile([128, 1], f32, tag="outsb")
    nc.vector.tensor_copy(outsb[:], out_ps[:])
    nc.sync.dma_start(outD[:], outsb[:])


def _build_program(st1, st2):
    nc = bacc.Bacc("TRN2", target_bir_lowering=False, debug=False, num_devices=NC)
    d = nc.dram_tensor
    identD = d("identity", [128, 128], f32, kind="ExternalInput")
    pool_loc = d("pool_loc", [GPC, 4736], f32)
    pool_glob = d("pool_glob", [G, 4736], f32, addr_space="Shared")

    with tile.TileContext(nc) as tc:
        with (
            tc.tile_pool(name="base", bufs=1) as base,
            tc.tile_pool(name="ps", bufs=2, space="PSUM") as ps,
        ):
            lib = nc.gpsimd.load_library(library_config.mlp)
            ident = base.tile([128, 128], f32, tag="ident")
            nc.sync.dma_start(ident[:], identD[:])
            identb = base.tile([128, 128], bf16, tag="identb")
            nc.vector.tensor_copy(identb[:], ident[:])
            pools = {"ps": ps, "ident": ident, "identb": identb, "lib": lib}

            b1 = Branch(nc, pools, "b1_", st1)
            b2 = Branch(nc, pools, "b2_", st2)
            b1.load_residents(base)
            b2.load_residents(base)

            # P1: T builds + GAT b1 aggregation
            with tc.tile_pool(name="p1", bufs=2) as sb:
                stg0 = sb.tile([GPC, 4736], f32, tag="stg0", bufs=1)
                nc.vector.memset(stg0[:], 0.0)
                nc.vector.memset(stg0[:, 4608:4609], 1.0)
                plw = nc.sync.dma_start(pool_loc[:], stg0[:])
                b1.emit_tbuild(sb)
                b2.emit_tbuild(sb)
                for b in range(st1["NBLK"]):
                    b1.emit_gat_block(sb, b)

            # P2: z-pass(b1) || GAT-agg(b2); AG_z(b1)
            with tc.tile_pool(name="p2", bufs=2) as sb:
                h1, h2 = _halves(st1["NBLK"])
                mid = st2["NBLK"] // 2
                b1.emit_zpass_half(sb, h1)
                for b in range(0, mid):
                    b2.emit_gat_block(sb, b)
                b1.emit_zpass_half(sb, h2)
                b1.emit_ag_z()
                for b in range(mid, st2["NBLK"]):
                    b2.emit_gat_block(sb, b)

            # P3: z-pass(b2) || GCN-agg(b1); AG_z(b2)
            with tc.tile_pool(name="p3", bufs=2) as sb:
                h1, h2 = _halves(st2["NBLK"])
                mid = st1["NBLK"] // 2
                b2.emit_zpass_half(sb, h1)
                for b in range(0, mid):
                    b1.emit_gcn_block(sb, b)
                b2.emit_zpass_half(sb, h2)
                b2.emit_ag_z()
                for b in range(mid, st1["NBLK"]):
                    b1.emit_gcn_block(sb, b)

            # P4: GCN-W(b1) + pooling(b1) || GCN-agg(b2)
            ws = []
            with tc.tile_pool(name="p4", bufs=2) as sb:
                b1.alloc_zfin(sb)
                h1, h2 = _halves(st1["NBLK"])
                mid = st2["NBLK"] // 2
                b1.emit_gcnw_half(sb, h1)
                for b in range(0, mid):
                    b2.emit_gcn_block(sb, b)
                b1.emit_gcnw_half(sb, h2)
                for b in range(mid, st2["NBLK"]):
                    b2.emit_gcn_block(sb, b)
                ws.extend(b1.emit_pooling(sb, pool_loc, 0, plw))

            # P5: GCN-W(b2) + pooling(b2)
            with tc.tile_pool(name="p5", bufs=2) as sb:
                b2.alloc_zfin(sb)
                for blks in _halves(st2["NBLK"]):
                    b2.emit_gcnw_half(sb, blks)
                ws.extend(b2.emit_pooling(sb, pool_loc, 2304, plw))

            ag_pool = nc.gpsimd.collective_compute(
                "AllGather", mybir.AluOpType.bypass,
                replica_groups=[list(range(NC))],
                ins=[pool_loc[:]], outs=[pool_glob[:]])
            add_dep_helper(ag_pool.ins, plw.ins, reason="AG pool after init")
            for w in ws:
                add_dep_helper(ag_pool.ins, w.ins, reason="AG pool after stages")
            with tc.tile_pool(name="tail", bufs=2) as sb:
                pools["sb"] = sb
                _build_tail(nc, pools, pool_glob, ag_pool)

    nc.compile()
    return nc


# ---------------------------------------------------------------- entry point
def kernel(**inputs) -> np.ndarray:
    st1, pc1, sh1 = _prep_branch(inputs["x1"], inputs["edge_index1"], inputs["batch1"],
                                 inputs["p1_gatW"], inputs["p1_att_src"],
                                 inputs["p1_att_dst"])
    st2, pc2, sh2 = _prep_branch(inputs["x2"], inputs["edge_index2"], inputs["batch2"],
                                 inputs["p2_gatW"], inputs["p2_att_src"],
                                 inputs["p2_att_dst"])
    gatWk1, gcnWk1 = _pack_branch_weights(inputs["p1_gatW"], inputs["p1_gatb"],
                                          inputs["p1_gcnW"], inputs["p1_gcnb"])
    gatWk2, gcnWk2 = _pack_branch_weights(inputs["p2_gatW"], inputs["p2_gatb"],
                                          inputs["p2_gcnW"], inputs["p2_gcnb"])
    tail = _pack_tail(inputs)

    key = (st1["MAXG"], st1["NSUBS"], st1["NGS"], st2["MAXG"], st2["NSUBS"],
           st2["NGS"])
    if key not in _PROG_CACHE:
        _PROG_CACHE[key] = _build_program(st1, st2)
    nc = _PROG_CACHE[key]

    in_maps = []
    for c in range(NC):
        m = {"identity": tail["identity"], "target": tail["target"],
             "fcxtWk": tail["fcxtWk"], "fc1Wk": tail["fc1Wk"],
             "fc2Wk": tail["fc2Wk"], "outWk": tail["outWk"],
             "p1_fcg1Wk": tail["p1_fcg1Wk"], "p1_fcg2Wk": tail["p1_fcg2Wk"],
             "p2_fcg1Wk": tail["p2_fcg1Wk"], "p2_fcg2Wk": tail["p2_fcg2Wk"]}
        for pfx, pc, sh, gatWk, gcnWk in (("b1_", pc1, sh1, gatWk1, gcnWk1),
                                          ("b2_", pc2, sh2, gatWk2, gcnWk2)):
            p = pc[c]
            m[pfx + "xslot"] = sh["xslot"]
            m[pfx + "xslotT"] = sh["xslotT"]
            m[pfx + "Wsd"] = sh["Wsd"]
            m[pfx + "gatWk"] = gatWk
            m[pfx + "gcnWk"] = gcnWk
            for k in ("isrc", "mask", "cols", "poison", "rcnt", "xownT"):
                m[pfx + k] = p[k]
        in_maps.append(m)

    res = run_bass_kernel_spmd(nc, in_maps, list(range(NC)))
    global LAST_RES
    LAST_RES = res
    return np.asarray(res.results[0]["out"], dtype=np.float32)


LAST_RES = None


# revision 36
# speedup vs baseline: 1.6322x; 1.0690x over previous
"""Trainium2 Bass kernel for nn_GAT_GCN (GAT conv + GCN conv + pooling + MLP tail).

Strategy (8 NeuronCores, SPMD, full inputs in / full output out), v3:
  - Nodes live in graph-aligned "slots" (MAXG per graph, 16 graphs per core);
    core c owns graphs [16c, 16c+16).  Edges are sharded by destination node
    and bucketed per 128-slot destination block; a host-built one-hot mask
    turns the per-destination segment-sum into TensorEngine matmuls.
  - Per-node tables T = [x | a_src] and T2 = [a_dst] (256 B bf16 rows) are
    built REPLICATED on every core from sequential reads of x (no collective,
    no index gathers).  Per-edge payloads come from dma_gathers of T[src] and
    T2[dst]; gather lengths are trimmed to the per-block max edge count
    (rounded to 16) instead of a global pad.
  - GAT: exp(lrelu(a_s+a_d)) per edge; softmax + symmetric-norm scaling are
    folded into the aggregated y tile (scaled by dinv/den per head on the
    Scalar engine) before the head-blocked weight multiply, which uses
    half-branch-batched DMA-transpose loads.  z := lrelu(dinv * GAT_out).
  - z is AllGathered (bf16) once per branch; the GCN aggregation gathers
    z[src] rows as the mask-matmul rhs; dinv[dst] applied on output.  The
    dense 1152x1152 multiply runs as a transposed pass producing zfin^T,
    which feeds max/mean pooling via free-dim reductions.
  - Emission is interleaved across branches at block granularity so that
    every engine queue always holds independent work:  z-pass(b1) overlaps
    GAT-agg(b2), z-pass(b2) overlaps GCN-agg(b1), GCN-W(b1) overlaps
    GCN-agg(b2), and each branch's z-AllGather overlaps the other branch's
    compute.  The tiny MLP tail is replicated on every core.

Host-side preprocessing is restricted to index manipulation (sorting/
bucketing edges, one-hot masks, padding, row permutations) and parameter
repacking (padding / bf16 casts / tiny reshapes) -- all data-dependent float
compute runs on device.
"""

import numpy as np
import ml_dtypes

import concourse.bacc as bacc
import concourse.tile as tile
from concourse import mybir, library_config
from concourse.bass_utils import run_bass_kernel_spmd
from concourse.tile_rust import add_dep_helper

# ---------------------------------------------------------------- constants
N = 20000
E = 160000
G = 128
F = 114
H = 10
HF = 1140          # F * H
NC = 8
GPC = G // NC      # graphs per core
P = 128

bf16 = mybir.dt.bfloat16
f32 = mybir.dt.float32
i16 = mybir.dt.int16

BF = ml_dtypes.bfloat16

_PROG_CACHE: dict = {}


# ---------------------------------------------------------------- host utils
def _wrap_idx(idx: np.ndarray) -> np.ndarray:
    """int16 index list (len % 16 == 0) -> [128, len/16] wrapped layout."""
    n = idx.shape[0]
    assert n % 16 == 0
    return np.tile(idx.reshape(-1, 16).T, (8, 1)).astype(np.int16)


def _prep_branch(x, ei, batch, gatW, att_src, att_dst):
    """Host preprocessing for one branch. Returns (static, percore, shared)."""
    x = np.asarray(x, dtype=np.float32)
    ei = np.asarray(ei).astype(np.int64)
    batch = np.asarray(batch).astype(np.int64)
    gatW = np.asarray(gatW, dtype=np.float32)
    att_src = np.asarray(att_src, dtype=np.float32)
    att_dst = np.asarray(att_dst, dtype=np.float32)

    cnt = np.bincount(batch, minlength=G)
    MAXG = int(cnt.max())
    SS = ((16 * MAXG + 127) // 128) * 128       # slots per core shard
    NBLK = SS // 128
    assert NC * SS <= 32768, "row ids must fit int16"

    gstart = np.zeros(G + 1, np.int64)
    np.cumsum(cnt, out=gstart[1:])
    nodes = np.arange(N)
    rank = nodes - gstart[batch]
    slot_of_node = (batch % GPC) * MAXG + rank          # [N] in [0, 16*MAXG)
    core_of_node = batch // GPC                          # [N]
    row_of_node = core_of_node * SS + slot_of_node       # [N] global table row

    src = np.concatenate([ei[0], nodes])
    dst = np.concatenate([ei[1], nodes])

    core_e = core_of_node[dst]
    slot_e = slot_of_node[dst]
    blk_e = slot_e // 128
    dloc_e = slot_e % 128

    # bucket edges per (core, block)
    order = np.lexsort((blk_e, core_e))
    src_s = src[order]
    dst_s = dst[order]
    blk_s, dloc_s = blk_e[order], dloc_e[order]
    key = core_e[order] * NBLK + blk_s
    counts = np.bincount(key, minlength=NC * NBLK)
    starts = np.zeros(NC * NBLK + 1, np.int64)
    np.cumsum(counts, out=starts[1:])
    co = counts.reshape(NC, NBLK)
    # exact subtile / gather-length per block, shared across cores
    maxcnt = co.max(axis=0)
    NSUBS = tuple(int(v) for v in np.maximum(1, -(-maxcnt // 128)))
    NGS = tuple(int(v) for v in np.maximum(16, 16 * (-(-maxcnt // 16))))
    TOTSUB = sum(NSUBS)
    TOTNG = sum(NGS)
    NSUBMAX = max(NSUBS)

    deg = np.bincount(dst, minlength=N).astype(np.float64)
    dinv_node = 1.0 / np.sqrt(np.maximum(deg, 1.0))

    slots = np.arange(SS)
    g_local = np.minimum(slots // MAXG, GPC - 1)
    r_local = slots - g_local * MAXG

    ngoff = np.zeros(NBLK + 1, np.int64)
    np.cumsum(NGS, out=ngoff[1:])

    percore = []
    for c in range(NC):
        isrc = np.zeros(TOTNG, np.int64)
        mask = np.zeros((128, TOTSUB, 128), np.float32)   # [e, (b s), d]
        soff = 0
        for b in range(NBLK):
            k = c * NBLK + b
            cnt_b = int(counts[k])
            sl = slice(starts[k], starts[k + 1])
            o = ngoff[b]
            isrc[o:o + cnt_b] = row_of_node[src_s[sl]]
            pos = np.arange(cnt_b)
            mask[pos % 128, soff + pos // 128, dloc_s[sl]] = 1.0
            soff += NSUBS[b]

        isrc_w = np.concatenate(
            [_wrap_idx(isrc[ngoff[b]:ngoff[b + 1]].astype(np.int16))
             for b in range(NBLK)], axis=1)

        g_global = c * GPC + g_local
        validity = (slots < 16 * MAXG) & (r_local < cnt[g_global])
        node_of_slot = np.zeros(SS, np.int64)
        real = validity.nonzero()[0]
        node_of_slot[real] = gstart[g_global[real]] + r_local[real]

        dinv_slot = np.ones(SS, np.float32)
        dinv_slot[real] = dinv_node[node_of_slot[real]]
        valid = validity.astype(np.float32)

        cols = np.zeros((128, NBLK, 4), np.float32)
        cols[:, :, 0] = dinv_slot.reshape(NBLK, 128).T
        cols[:, :, 1] = (1.0 - valid).reshape(NBLK, 128).T
        cols[:, :, 2] = valid.reshape(NBLK, 128).T

        poison = np.where(validity, 0.0, -1e28).astype(np.float32)
        rcnt = (1.0 / np.maximum(cnt[c * GPC:(c + 1) * GPC], 1.0)).astype(np.float32)

        percore.append({
            "isrc": isrc_w,                                          # [128, TOTNG/16]
            "mask": mask.astype(BF),                                 # [128, TOTSUB, 128]
            "cols": cols,                                            # [128, NBLK, 4]
            "poison": np.broadcast_to(poison.astype(BF), (128, SS)).copy(),
            "rcnt": np.broadcast_to(rcnt, (128, GPC)).copy(),
        })

    # shared tables: slot-ordered x (replicated T build), plus its transpose
    xslot = np.zeros((NC * SS, 128), np.float32)
    for c in range(NC):
        g_global = c * GPC + g_local
        validity = (slots < 16 * MAXG) & (r_local < cnt[g_global])
        real = validity.nonzero()[0]
        node = gstart[g_global[real]] + r_local[real]
        xslot[c * SS + real, :F] = x[node]
    xslot_bf = xslot.astype(BF)
    for c in range(NC):
        percore[c]["xownT"] = np.ascontiguousarray(
            xslot_bf[c * SS:(c + 1) * SS].T)                # [128, SS]

    gatW3 = gatW.reshape(F, H, F)
    Ws = np.einsum("khf,hf->kh", gatW3, att_src)
    Wd = np.einsum("khf,hf->kh", gatW3, att_dst)
    Wsd = np.zeros((128, 2 * H), np.float32)
    Wsd[:F, :H] = Ws
    Wsd[:F, H:] = Wd

    shared = {
        "xslot": xslot_bf,                                  # [NC*SS, 128]
        "xslotT": np.ascontiguousarray(xslot_bf.T),         # [128, NC*SS]
        "Wsd": Wsd.astype(BF),                              # [128, 20]
    }
    static = dict(MAXG=MAXG, SS=SS, NBLK=NBLK, NSUBS=NSUBS, NGS=NGS,
                  TOTSUB=TOTSUB, TOTNG=TOTNG, NSUBMAX=NSUBMAX)
    return static, percore, shared


def _pack_branch_weights(gatW, gatb, gcnW, gcnb):
    gatW = np.asarray(gatW, np.float32)
    gatb = np.asarray(gatb, np.float32)
    gcnW = np.asarray(gcnW, np.float32)
    gcnb = np.asarray(gcnb, np.float32)
    gatWk = np.zeros((H, 128, F), np.float32)
    gatW3 = gatW.reshape(F, H, F)
    for h in range(H):
        gatWk[h, :F, :] = gatW3[:, h, :]
        gatWk[h, F, :] = gatb[h * F:(h + 1) * F]
    gcn_pad = np.zeros((1152, 1152), np.float32)
    gcn_pad[:HF, :HF] = gcnW
    gcn_pad[HF, :HF] = gcnb
    gcnWk = gcn_pad.reshape(9, 128, 1152)
    return gatWk.astype(BF), gcnWk.astype(BF)


def _pack_tail(inp):
    f = lambda k: np.asarray(inp[k], np.float32)
    tail = {}
    for p in ("p1", "p2"):
        W1 = np.zeros((2432, 1024), np.float32)
        fg1 = f(p + "_fcg1W")           # [2280, 1000]
        W1[0:HF, 0:1000] = fg1[0:HF]
        W1[1152:1152 + HF, 0:1000] = fg1[HF:2 * HF]
        k1 = np.concatenate([W1[:2304].reshape(18, 128, 1024),
                             np.zeros((1, 128, 1024), np.float32)], axis=0)
        k1[18, 0, 0:1000] = f(p + "_fcg1b")
        tail[p + "_fcg1Wk"] = k1.astype(np.float32)
        W2 = np.zeros((1024, 64), np.float32)
        W2[0:1000] = f(p + "_fcg2W")
        W2[1000] = f(p + "_fcg2b")
        tail[p + "_fcg2Wk"] = W2.reshape(8, 128, 64).astype(np.float32)
    Wx = np.zeros((1024, 128), np.float32)
    Wx[0:1000] = f("fcxtW")
    Wx[1000] = f("fcxtb")
    tail["fcxtWk"] = Wx.reshape(8, 128, 128).astype(np.float32)
    W1 = np.zeros((3, 128, 128), np.float32)
    W1[0] = f("fc1W")[0:128]
    W1[1] = f("fc1W")[128:256]
    W1[2, 0] = f("fc1b")
    tail["fc1Wk"] = W1.astype(np.float32)
    W2 = np.zeros((2, 128, 32), np.float32)
    W2[0] = f("fc2W")
    W2[1, 0] = f("fc2b")
    tail["fc2Wk"] = W2.astype(np.float32)
    Wo = np.zeros((128, 1), np.float32)
    Wo[0:32, 0] = f("outW")[:, 0]
    Wo[32, 0] = float(np.asarray(inp["outb"]).reshape(-1)[0])
    tail["outWk"] = Wo.astype(np.float32)
    tail["identity"] = np.eye(128, dtype=np.float32)
    tail["target"] = f("target")
    return tail


# ---------------------------------------------------------------- device build
GATHER_MAX = 1024  # dma_gather breaks above 1024 indices per call


class Branch:
    """Per-branch emission state: DRAM handles, resident tiles, generators."""

    def __init__(self, nc, pools, pfx, st):
        self.nc = nc
        self.pools = pools
        self.pfx = pfx
        self.st = st
        MAXG, SS, NBLK = st["MAXG"], st["SS"], st["NBLK"]
        TOTSUB, TOTNG = st["TOTSUB"], st["TOTNG"]
        d = nc.dram_tensor
        self.xslotD = d(pfx + "xslot", [NC * SS, 128], bf16, kind="ExternalInput")
        self.xslotTD = d(pfx + "xslotT", [128, NC * SS], bf16, kind="ExternalInput")
        self.xownTD = d(pfx + "xownT", [128, SS], bf16, kind="ExternalInput")
        self.WsdD = d(pfx + "Wsd", [128, 2 * H], bf16, kind="ExternalInput")
        self.isrcD = d(pfx + "isrc", [128, TOTNG // 16], i16, kind="ExternalInput")
        self.maskD = d(pfx + "mask", [128, TOTSUB, 128], bf16, kind="ExternalInput")
        self.colsD = d(pfx + "cols", [128, NBLK, 4], f32, kind="ExternalInput")
        self.gatWkD = d(pfx + "gatWk", [H, 128, F], bf16, kind="ExternalInput")
        self.gcnWkD = d(pfx + "gcnWk", [9, 128, 1152], bf16, kind="ExternalInput")
        self.poisonD = d(pfx + "poison", [128, SS], bf16, kind="ExternalInput")
        self.rcntD = d(pfx + "rcnt", [128, GPC], f32, kind="ExternalInput")
        self.TD = d(pfx + "T", [NC * SS, 128], bf16)
        self.y_dram = d(pfx + "y", [SS, 1280], bf16)
        self.z_loc = d(pfx + "z_loc", [SS, 1152], bf16)
        self.z_glob = d(pfx + "z_glob", [NC * SS, 1152], bf16, addr_space="Shared")
        self.y2_dram = d(pfx + "y2", [SS, 1152], bf16)

        self.fence = None
        self.y_writes = []
        self.z_writes = []
        self.y2_writes = []
        self.ag_z = None

        # offsets
        self.soffs = np.zeros(NBLK + 1, np.int64)
        np.cumsum(st["NSUBS"], out=self.soffs[1:])
        self.ioffs = np.zeros(NBLK + 1, np.int64)
        np.cumsum([n // 16 for n in st["NGS"]], out=self.ioffs[1:])

    # ---- resident tiles (call once inside the long-lived base pool) ----
    def load_residents(self, base):
        nc = self.nc
        self.gatWk = base.tile([128, H, F], bf16, tag=self.pfx + "gatwk")
        nc.sync.dma_start(self.gatWk[:], self.gatWkD.ap().rearrange("h k n -> k h n"))
        self.cols = base.tile([128, self.st["NBLK"], 4], f32, tag=self.pfx + "cols")
        nc.sync.dma_start(self.cols[:], self.colsD[:])
        self.ii = base.tile([128, self.st["TOTNG"] // 16], i16, tag=self.pfx + "ii")
        nc.sync.dma_start(self.ii[:], self.isrcD[:])
        self.Wsd = base.tile([128, 2 * H], bf16, tag=self.pfx + "wsd")
        nc.sync.dma_start(self.Wsd[:], self.WsdD[:])
        self.xownT = base.tile([128, self.st["SS"]], bf16, tag=self.pfx + "xownT")
        nc.sync.dma_start(self.xownT[:], self.xownTD[:])

    def _gather(self, out_tile, table_ap, idx_tile, b, elem, deps):
        nc, pools = self.nc, self.pools
        ng = self.st["NGS"][b]
        i0 = int(self.ioffs[b])
        if b < 2:
            # fully zero each tag's two buffers once; afterwards stale rows
            # always hold old finite data, so trailing -1 (skipped) indices
            # and partial subtiles are safe
            nc.vector.memset(out_tile[:], 0.0)
        insts = []
        for o in range(0, ng, GATHER_MAX):
            n = min(ng, o + GATHER_MAX) - o
            g = nc.gpsimd.dma_gather(
                out_tile[:, o // 128:(o + n + 127) // 128, :], table_ap,
                idx_tile[:, i0 + o // 16:i0 + (o + n) // 16], n, n, elem)
            add_dep_helper(g.ins, pools["lib"].ins, reason="gather after lib")
            for dd in deps:
                add_dep_helper(g.ins, dd.ins, reason="gather dep")
            insts.append(g)
        return insts

    # ---- T build ----
    def emit_tbuild(self, sb):
        nc, ps = self.nc, self.pools["ps"]
        writes = []
        TOT = NC * self.st["SS"]
        for r0 in range(0, TOT, 2048):
            rows = min(2048, TOT - r0)
            na = rows // 128
            xt = sb.tile([128, 2048], bf16, tag="xt")
            nc.sync.dma_start(xt[:, 0:rows], self.xslotTD[:, r0:r0 + rows])
            xr = sb.tile([128, 16, 128], bf16, tag="xr")
            nc.sync.dma_start(
                xr[:, 0:na, :],
                self.xslotD[r0:r0 + rows, :].rearrange("(a p) c -> p a c", p=128))
            aps = ps.tile([128, 16, H], f32, tag="ps_small")
            for a in range(na):
                nc.tensor.matmul(aps[:, a, :], xt[:, a * 128:(a + 1) * 128],
                                 self.Wsd[:, 0:H], start=True, stop=True)
            Tt = sb.tile([128, 16, 128], bf16, tag="Tt")
            nc.vector.tensor_copy(Tt[:, 0:na, 0:F], xr[:, 0:na, 0:F])
            nc.vector.tensor_copy(Tt[:, 0:na, F:F + H], aps[:, 0:na, :])
            nc.vector.memset(Tt[:, 0:na, F + H:128], 0.0)
            w1 = nc.sync.dma_start(
                self.TD[r0:r0 + rows, :].rearrange("(a p) c -> p a c", p=128),
                Tt[:, 0:na, :])
            writes.append(w1)
        fence_t = sb.tile([128, 1], f32, tag="fence" + self.pfx)
        self.fence = nc.vector.memset(fence_t[:], 0.0)
        for w in writes:
            add_dep_helper(self.fence.ins, w.ins, reason="T fence")

    # ---- GAT aggregation, one block ----
    def prezero(self, sb, specs):
        """One-time zeroing of both buffers of the given (shape, dtype, tag)s
        so later partially-written tiles never expose uninitialized memory."""
        nc = self.nc
        for shape, dtype, tag in specs:
            for _ in range(2):
                t = sb.tile(shape, dtype, tag=tag)
                nc.vector.memset(t[:], 0.0)

    def emit_gat_block(self, sb, b):
        nc, ps = self.nc, self.pools["ps"]
        ns = self.st["NSUBS"][b]
        NSM = self.st["NSUBMAX"]
        soff = int(self.soffs[b])
        cols = self.cols

        S = sb.tile([128, NSM, 128], bf16, tag="S")
        self._gather(S, self.TD[:], self.ii, b, 128, (self.fence,))
        Mt = sb.tile([128, NSM, 128], bf16, tag="Mt")
        nc.sync.dma_start(Mt[:, 0:ns, :], self.maskD[:, soff:soff + ns, :])

        # a_d of this block's 128 destination slots, from local x^T
        y_ps = ps.tile([128, 1280], f32, tag="ps_big")
        adb_ps = ps.tile([128, H], f32, tag="ps_small")
        nc.tensor.matmul(adb_ps[:], self.xownT[:, b * 128:(b + 1) * 128],
                         self.Wsd[:, H:2 * H], start=True, stop=True)
        adb = sb.tile([128, H], bf16, tag="adb")
        nc.scalar.copy(adb[:], adb_ps[:])

        # broadcast a_d[dst] to edge rows via transposed-mask matmuls, all
        # subtiles batched into spare y_ps columns (no per-subtile V sync)
        identb = self.pools["identb"]
        for s in range(ns):
            mtt_ps = ps.tile([128, 128], bf16, tag="ps_small")
            nc.tensor.transpose(mtt_ps[:], Mt[:, s, :], identb[:])
            mtt = sb.tile([128, 128], bf16, tag="mtt")
            nc.scalar.copy(mtt[:], mtt_ps[:])
            nc.tensor.matmul(y_ps[:, 1150 + s * H:1150 + (s + 1) * H],
                             mtt[:], adb[:], start=True, stop=True)
        lg = sb.tile([128, NSM, H], f32, tag="lg")
        nc.vector.tensor_tensor(
            out=lg[:, 0:ns, :], in0=S[:, 0:ns, F:F + H],
            in1=y_ps[:, 1150:1150 + ns * H].rearrange("p (s h) -> p s h", h=H),
            op=mybir.AluOpType.add)
        l3 = sb.tile([128, NSM, H], f32, tag="l3")
        nc.vector.scalar_tensor_tensor(out=l3[:, 0:ns, :], in0=lg[:, 0:ns, :],
                                       scalar=0.2, in1=lg[:, 0:ns, :],
                                       op0=mybir.AluOpType.mult,
                                       op1=mybir.AluOpType.max)
        exb = sb.tile([128, NSM, H], bf16, tag="exb")
        nc.scalar.activation(exb[:, 0:ns, :], l3[:, 0:ns, :],
                             mybir.ActivationFunctionType.Exp)

        R = sb.tile([128, NSM, 1152], bf16, tag="R")
        nc.vector.tensor_tensor(
            out=R[:, 0:ns, 0:HF].rearrange("p s (h f) -> p s h f", h=H),
            in0=S[:, 0:ns, 0:F].unsqueeze(2).broadcast_to([128, ns, H, F]),
            in1=exb[:, 0:ns, :].unsqueeze(3).broadcast_to([128, ns, H, F]),
            op=mybir.AluOpType.mult)
        nc.scalar.copy(R[:, 0:ns, HF:1150], exb[:, 0:ns, :])

        for s in range(ns):
            for c0, c1 in ((0, 512), (512, 1024), (1024, 1150)):
                nc.tensor.matmul(y_ps[:, c0:c1], Mt[:, s, :], R[:, s, c0:c1],
                                 start=(s == 0), stop=(s == ns - 1))

        den = sb.tile([128, H], f32, tag="den")
        nc.scalar.activation(den[:], y_ps[:, HF:1150],
                             mybir.ActivationFunctionType.Lrelu,
                             bias=cols[:, b, 1:2], alpha=1.0)
        rden = sb.tile([128, H], f32, tag="rden")
        nc.vector.reciprocal(rden[:], den[:])
        rdn = sb.tile([128, H], f32, tag="rdn")
        nc.vector.tensor_scalar(out=rdn[:], in0=rden[:], scalar1=cols[:, b, 0:1],
                                scalar2=None, op0=mybir.AluOpType.mult)

        y_sb = sb.tile([128, 1280], bf16, tag="ysb")
        ytv = y_sb[:].rearrange("p (h c) -> p h c", h=H)
        ypv = y_ps[:, 0:HF].rearrange("p (h f) -> p h f", h=H)
        for h in range(H):
            nc.scalar.activation(ytv[:, h, 0:F], ypv[:, h, :],
                                 mybir.ActivationFunctionType.Copy,
                                 scale=rdn[:, h:h + 1])
        nc.vector.tensor_copy(
            ytv[:, :, F:F + 1],
            cols[:, b, 0:1].unsqueeze(1).broadcast_to([128, H, 1]))
        w = nc.sync.dma_start(self.y_dram[b * 128:(b + 1) * 128, :], y_sb[:])
        self.y_writes.append(w)

    # ---- z production: one half (list of blocks), transposes batched ----
    def emit_zpass_half(self, sb, blocks):
        nc, ps = self.nc, self.pools["ps"]
        t0, t1 = blocks[0], blocks[-1] + 1
        rows = (t1 - t0) * 128
        yTs = []
        for h in range(H):
            yT = sb.tile([128, 13 * 128], bf16, tag=f"yT{h}", bufs=1)
            ld = nc.sync.dma_start_transpose(
                out=yT[:, 0:rows],
                in_=self.y_dram[t0 * 128:t1 * 128, h * 128:(h + 1) * 128])
            for t in blocks:
                add_dep_helper(ld.ins, self.y_writes[t].ins, reason="yT after y")
            yTs.append(yT)
        for t in blocks:
            o = (t - t0) * 128
            zp = ps.tile([128, 1280], f32, tag="ps_big")
            for h in range(H):
                nc.tensor.matmul(zp[:, h * 128:h * 128 + F],
                                 yTs[h][:, o:o + 128], self.gatWk[:, h, :],
                                 start=(h % 4 == 0), stop=(h % 4 == 3) or (h == H - 1))
            zt = sb.tile([128, 1152], bf16, tag="zt")
            nc.scalar.activation(
                zt[:, 0:HF].rearrange("p (h f) -> p h f", h=H),
                zp[:].rearrange("p (h c) -> p h c", h=H)[:, :, 0:F],
                mybir.ActivationFunctionType.Lrelu, alpha=0.01)
            w = nc.sync.dma_start(self.z_loc[t * 128:(t + 1) * 128, :], zt[:])
            self.z_writes.append(w)

    def emit_ag_z(self):
        nc = self.nc
        self.ag_z = nc.gpsimd.collective_compute(
            "AllGather", mybir.AluOpType.bypass,
            replica_groups=[list(range(NC))],
            ins=[self.z_loc[:]], outs=[self.z_glob[:]])
        for w in self.z_writes:
            add_dep_helper(self.ag_z.ins, w.ins, reason="AG_z after z writes")

    # ---- GCN aggregation, one block ----
    def emit_gcn_block(self, sb, b):
        nc, ps = self.nc, self.pools["ps"]
        ns = self.st["NSUBS"][b]
        NSM = self.st["NSUBMAX"]
        soff = int(self.soffs[b])
        cols = self.cols

        Z = sb.tile([128, NSM, 1152], bf16, tag="Z")
        self._gather(Z, self.z_glob[:], self.ii, b, 1152, (self.ag_z,))
        Mt = sb.tile([128, NSM, 128], bf16, tag="Mt2")
        nc.sync.dma_start(Mt[:, 0:ns, :], self.maskD[:, soff:soff + ns, :])

        y2_ps = ps.tile([128, 1280], f32, tag="ps_big")
        for s in range(ns):
            for c0, c1 in ((0, 512), (512, 1024), (1024, HF)):
                nc.tensor.matmul(y2_ps[:, c0:c1], Mt[:, s, :], Z[:, s, c0:c1],
                                 start=(s == 0), stop=(s == ns - 1))
        y2t = sb.tile([128, 1152], bf16, tag="y2t")
        nc.scalar.activation(y2t[:, 0:512], y2_ps[:, 0:512],
                             mybir.ActivationFunctionType.Copy,
                             scale=cols[:, b, 0:1])
        nc.scalar.activation(y2t[:, 512:HF], y2_ps[:, 512:HF],
                             mybir.ActivationFunctionType.Copy,
                             scale=cols[:, b, 0:1])
        nc.vector.tensor_copy(y2t[:, HF:HF + 1], cols[:, b, 2:3])
        w = nc.sync.dma_start(self.y2_dram[b * 128:(b + 1) * 128, :], y2t[:])
        self.y2_writes.append(w)

    # ---- GCN W-pass: one half (transposes batched), writes zfin slices ----
    def alloc_zfin(self, sb):
        nc = self.nc
        self.gcnWk = sb.tile([128, 9, 1152], bf16, tag=self.pfx + "gcnwk", bufs=1)
        nc.sync.dma_start(self.gcnWk[:],
                          self.gcnWkD.ap().rearrange("kb kr n -> kr kb n"))
        self.zfin = sb.tile([128, 9, self.st["SS"]], bf16,
                            tag=self.pfx + "zfin", bufs=1)

    def emit_gcnw_half(self, sb, blocks):
        nc, ps = self.nc, self.pools["ps"]
        t0, t1 = blocks[0], blocks[-1] + 1
        rows = (t1 - t0) * 128
        r0 = t0 * 128
        yTs = []
        for kb in range(9):
            y2T = sb.tile([128, 13 * 128], bf16, tag=f"y2T{kb}", bufs=1)
            ld = nc.sync.dma_start_transpose(
                out=y2T[:, 0:rows],
                in_=self.y2_dram[r0:r0 + rows, kb * 128:(kb + 1) * 128])
            for t in blocks:
                add_dep_helper(ld.ins, self.y2_writes[t].ins, reason="y2T dep")
            yTs.append(y2T)
        for c0 in range(0, rows, 512):
            cw = min(512, rows - c0)
            for nb in range(9):
                ct = ps.tile([128, 512], f32, tag="ps_small")
                for kb in range(9):
                    nc.tensor.matmul(ct[:, 0:cw],
                                     self.gcnWk[:, kb, nb * 128:(nb + 1) * 128],
                                     yTs[kb][:, c0:c0 + cw],
                                     start=(kb == 0), stop=(kb == 8))
                nc.scalar.activation(self.zfin[:, nb, r0 + c0:r0 + c0 + cw],
                                     ct[:, 0:cw],
                                     mybir.ActivationFunctionType.Lrelu, alpha=0.01)

    # ---- pooling + staging ----
    def emit_pooling(self, sb, pool_loc, pool_col0, plw):
        nc, ps = self.nc, self.pools["ps"]
        MAXG = self.st["MAXG"]
        ident = self.pools["ident"]
        poison = sb.tile([128, self.st["SS"]], bf16, tag=self.pfx + "poison", bufs=1)
        nc.sync.dma_start(poison[:], self.poisonD[:])
        rcnt = sb.tile([128, GPC], f32, tag=self.pfx + "rcnt")
        nc.sync.dma_start(rcnt[:], self.rcntD[:])
        mxT = sb.tile([128, 9, GPC], f32, tag=self.pfx + "mxT")
        smT = sb.tile([128, 9, GPC], f32, tag=self.pfx + "smT")
        for g in range(GPC):
            s0 = g * MAXG
            tmp = sb.tile([128, 9, MAXG], bf16, tag="ptmp")
            nc.vector.tensor_tensor(
                out=tmp[:], in0=self.zfin[:, :, s0:s0 + MAXG],
                in1=poison[:, s0:s0 + MAXG].unsqueeze(1).broadcast_to([128, 9, MAXG]),
                op=mybir.AluOpType.add)
            nc.vector.reduce_max(mxT[:, :, g:g + 1], tmp[:],
                                 axis=mybir.AxisListType.X)
            nc.vector.reduce_sum(smT[:, :, g:g + 1], self.zfin[:, :, s0:s0 + MAXG],
                                 axis=mybir.AxisListType.X)
        mnT = sb.tile([128, 9, GPC], f32, tag=self.pfx + "mnT")
        nc.vector.tensor_tensor(out=mnT[:], in0=smT[:],
                                in1=rcnt[:].unsqueeze(1).broadcast_to([128, 9, GPC]),
                                op=mybir.AluOpType.mult)
        writes = []
        for which, statT in ((0, mxT), (1, mnT)):
            for ft in range(9):
                tp = ps.tile([GPC, 128], f32, tag="ps_small")
                nc.tensor.transpose(tp[:], statT[:, ft, :], ident[:])
                stg = sb.tile([GPC, 128], f32, tag="stg")
                nc.vector.tensor_copy(stg[:], tp[:])
                w = nc.sync.dma_start(
                    pool_loc[:, pool_col0 + which * 1152 + ft * 128:
                             pool_col0 + which * 1152 + ft * 128 + 128], stg[:])
                add_dep_helper(w.ins, plw.ins, reason="stage after pool init")
                writes.append(w)
        return writes


def _halves(NBLK):
    h = (NBLK + 1) // 2
    return [list(range(0, h)), list(range(h, NBLK))]


def _build_tail(nc, pools, globs, ag_pools):
    d = nc.dram_tensor
    sb, ps = pools["sb"], pools["ps"]
    ident = pools["ident"]
    tgtD = d("target", [G, 1000], f32, kind="ExternalInput")
    fcxtWkD = d("fcxtWk", [8, 128, 128], f32, kind="ExternalInput")
    fc1WkD = d("fc1Wk", [3, 128, 128], f32, kind="ExternalInput")
    fc2WkD = d("fc2Wk", [2, 128, 32], f32, kind="ExternalInput")
    outWkD = d("outWk", [128, 1], f32, kind="ExternalInput")
    outD = d("out", [G, 1], f32, kind="ExternalOutput")

    def pe_T(src_ap, rows):
        tp = ps.tile([rows, 128], f32, tag="ps_small")
        nc.tensor.transpose(tp[:], src_ap, ident[:])
        return tp

    def mm_transposed(src_tile, nk, rhs_fn, psum, chunks, tag):
        for k in range(nk):
            tp = pe_T(src_tile[:, k * 128:(k + 1) * 128], 128)
            tt = sb.tile([128, 128], f32, tag=tag)
            nc.vector.tensor_copy(tt[:], tp[:])
            for c0, c1 in chunks:
                nc.tensor.matmul(psum[:, c0:c1], tt[:], rhs_fn(k)[:, c0:c1],
                                 start=(k == 0), stop=(k == nk - 1))

    tg = sb.tile([128, 1024], f32, tag="tg")
    nc.sync.dma_start(tg[:, 0:1000], tgtD[:])
    nc.vector.memset(tg[:, 1000:1001], 1.0)
    nc.vector.memset(tg[:, 1001:1024], 0.0)
    fcxtWk = sb.tile([128, 8, 128], f32, tag="tw8")
    nc.sync.dma_start(fcxtWk[:], fcxtWkD.ap().rearrange("k r n -> r k n"))
    xt_ps = ps.tile([128, 128], f32, tag="ps_small")
    mm_transposed(tg, 8, lambda k: fcxtWk[:, k, :], xt_ps, ((0, 128),), "ttl")
    xt_sb = sb.tile([128, 128], f32, tag="xt2")
    nc.vector.tensor_copy(xt_sb[:], xt_ps[:])

    gvecs = []
    for bi, p in enumerate(("p1", "p2")):
        fg1D = d(p + "_fcg1Wk", [19, 128, 1024], f32, kind="ExternalInput")
        fg2D = d(p + "_fcg2Wk", [8, 128, 64], f32, kind="ExternalInput")
        fg1 = sb.tile([128, 19, 1024], f32, tag="fg1", bufs=1)
        nc.sync.dma_start(fg1[:], fg1D.ap().rearrange("k r n -> r k n"))
        g_ps = ps.tile([128, 1024], f32, tag="ps_big")
        for k, kt in enumerate(range(19)):
            pl0 = sb.tile([128, 128], f32, tag="pl0")
            ld = nc.sync.dma_start(pl0[:], globs[bi][:, kt * 128:(kt + 1) * 128])
            add_dep_helper(ld.ins, ag_pools[bi].ins, reason="pool load after AG")
            tp = pe_T(pl0[:], 128)
            pl = sb.tile([128, 128], f32, tag="plt")
            nc.vector.tensor_copy(pl[:], tp[:])
            for c0, c1 in ((0, 512), (512, 1024)):
                nc.tensor.matmul(g_ps[:, c0:c1], pl[:], fg1[:, k, c0:c1],
                                 start=(k == 0), stop=(k == 18))
        glr = sb.tile([128, 1024], f32, tag="glr")
        nc.scalar.activation(glr[:, 0:1000], g_ps[:, 0:1000],
                             mybir.ActivationFunctionType.Lrelu, alpha=0.01)
        nc.vector.memset(glr[:, 1000:1001], 1.0)
        nc.vector.memset(glr[:, 1001:1024], 0.0)
        fg2 = sb.tile([128, 8, 64], f32, tag="tw8b")
        nc.sync.dma_start(fg2[:], fg2D.ap().rearrange("k r n -> r k n"))
        g2_ps = ps.tile([128, 64], f32, tag="ps_small")
        mm_transposed(glr, 8, lambda k: fg2[:, k, :], g2_ps, ((0, 64),), "gtl")
        gv = sb.tile([128, 64], f32, tag=f"gv{bi}")
        nc.vector.tensor_copy(gv[:], g2_ps[:])
        gvecs.append(gv)

    xcT0 = sb.tile([128, 128], f32, tag="xcT0")
    t0 = pe_T(gvecs[0][:], 64)
    nc.vector.tensor_copy(xcT0[0:64, :], t0[:])
    t1 = pe_T(gvecs[1][:], 64)
    nc.vector.tensor_copy(xcT0[64:128, :], t1[:])
    xcT1 = sb.tile([128, 128], f32, tag="xcT1")
    t2 = pe_T(xt_sb[:], 128)
    nc.vector.tensor_copy(xcT1[:], t2[:])
    ones = sb.tile([128, 128], f32, tag="ones")
    nc.vector.memset(ones[:], 0.0)
    nc.vector.memset(ones[0:1, :], 1.0)

    fc1Wk = sb.tile([128, 3, 128], f32, tag="fc1w")
    nc.sync.dma_start(fc1Wk[:], fc1WkD.ap().rearrange("k r n -> r k n"))
    xc1_ps = ps.tile([128, 128], f32, tag="ps_small")
    for k, lt in enumerate((xcT0, xcT1, ones)):
        nc.tensor.matmul(xc1_ps[:], lt[:], fc1Wk[:, k, :], start=(k == 0), stop=(k == 2))
    xc1 = sb.tile([128, 128], f32, tag="xc1")
    nc.scalar.activation(xc1[:], xc1_ps[:],
                         mybir.ActivationFunctionType.Lrelu, alpha=0.01)
    xc1T = sb.tile([128, 128], f32, tag="xc1T")
    t3 = pe_T(xc1[:], 128)
    nc.vector.tensor_copy(xc1T[:], t3[:])

    fc2Wk = sb.tile([128, 2, 32], f32, tag="fc2w")
    nc.sync.dma_start(fc2Wk[:], fc2WkD.ap().rearrange("k r n -> r k n"))
    xc2_ps = ps.tile([128, 32], f32, tag="ps_small")
    for k, lt in enumerate((xc1T, ones)):
        nc.tensor.matmul(xc2_ps[:], lt[:], fc2Wk[:, k, :], start=(k == 0), stop=(k == 1))
    xc2 = sb.tile([128, 32], f32, tag="xc2")
    nc.scalar.activation(xc2[:], xc2_ps[:],
                         mybir.ActivationFunctionType.Lrelu, alpha=0.01)
    xc2T = sb.tile([128, 128], f32, tag="xc2T")
    nc.vector.memset(xc2T[:], 0.0)
    t4 = pe_T(xc2[:], 32)
    nc.vector.tensor_copy(xc2T[0:32, :], t4[:])
    nc.vector.memset(xc2T[32:33, :], 1.0)

    outWk = sb.tile([128, 1], f32, tag="outw")
    nc.sync.dma_start(outWk[:], outWkD[:])
    out_ps = ps.tile([128, 1], f32, tag="ps_small")
    nc.tensor.matmul(out_ps[:], xc2T[:], outWk[:], start=True, stop=True)
    outsb = sb.tile([128, 1], f32, tag="outsb")
    nc.vector.tensor_copy(outsb[:], out_ps[:])
    nc.sync.dma_start(outD[:], outsb[:])


def _build_program(st1, st2):
    nc = bacc.Bacc("TRN2", target_bir_lowering=False, debug=False, num_devices=NC)
    d = nc.dram_tensor
    identD = d("identity", [128, 128], f32, kind="ExternalInput")
    pool_loc1 = d("pool_loc1", [GPC, 2432], f32)
    pool_glob1 = d("pool_glob1", [G, 2432], f32, addr_space="Shared")
    pool_loc2 = d("pool_loc2", [GPC, 2432], f32)
    pool_glob2 = d("pool_glob2", [G, 2432], f32, addr_space="Shared")

    with tile.TileContext(nc) as tc:
        with (
            tc.tile_pool(name="base", bufs=1) as base,
            tc.tile_pool(name="ps", bufs=2, space="PSUM") as ps,
        ):
            lib = nc.gpsimd.load_library(library_config.mlp)
            ident = base.tile([128, 128], f32, tag="ident")
            nc.sync.dma_start(ident[:], identD[:])
            identb = base.tile([128, 128], bf16, tag="identb")
            nc.vector.tensor_copy(identb[:], ident[:])
            pools = {"ps": ps, "ident": ident, "identb": identb, "lib": lib}

            b1 = Branch(nc, pools, "b1_", st1)
            b2 = Branch(nc, pools, "b2_", st2)
            b1.load_residents(base)
            b2.load_residents(base)

            # P1: T builds + GAT b1 aggregation
            with tc.tile_pool(name="p1", bufs=2) as sb:
                plws = []
                for i, pl in enumerate((pool_loc1, pool_loc2)):
                    stg0 = sb.tile([GPC, 2432], f32, tag=f"stg{i}", bufs=1)
                    nc.vector.memset(stg0[:], 0.0)
                    nc.vector.memset(stg0[:, 2304:2305], 1.0)
                    plws.append(nc.sync.dma_start(pl[:], stg0[:]))
                b1.prezero(sb, [([128, 1280], bf16, "ysb")])
                b1.emit_tbuild(sb)
                b2.emit_tbuild(sb)
                for b in range(st1["NBLK"]):
                    b1.emit_gat_block(sb, b)

            # P2: z-pass(b1) || GAT-agg(b2); AG_z(b1)
            with tc.tile_pool(name="p2", bufs=2) as sb:
                b2.prezero(sb, [([128, 1280], bf16, "ysb")])
                b1.prezero(sb, [([128, 1152], bf16, "zt")])
                h1, h2 = _halves(st1["NBLK"])
                mid = st2["NBLK"] // 2
                b1.emit_zpass_half(sb, h1)
                for b in range(0, mid):
                    b2.emit_gat_block(sb, b)
                b1.emit_zpass_half(sb, h2)
                b1.emit_ag_z()
                for b in range(mid, st2["NBLK"]):
                    b2.emit_gat_block(sb, b)

            # P3: z-pass(b2) || GCN-agg(b1); AG_z(b2)
            with tc.tile_pool(name="p3", bufs=2) as sb:
                b2.prezero(sb, [([128, 1152], bf16, "zt")])
                b1.prezero(sb, [([128, 1152], bf16, "y2t")])
                h1, h2 = _halves(st2["NBLK"])
                mid = st1["NBLK"] // 2
                b2.emit_zpass_half(sb, h1)
                for b in range(0, mid):
                    b1.emit_gcn_block(sb, b)
                b2.emit_zpass_half(sb, h2)
                b2.emit_ag_z()
                for b in range(mid, st1["NBLK"]):
                    b1.emit_gcn_block(sb, b)

            # P4: GCN-W(b1) + pooling(b1) || GCN-agg(b2); AG_pool(b1)
            with tc.tile_pool(name="p4", bufs=2) as sb:
                b2.prezero(sb, [([128, 1152], bf16, "y2t")])
                b1.alloc_zfin(sb)
                h1, h2 = _halves(st1["NBLK"])
                mid = st2["NBLK"] // 2
                b1.emit_gcnw_half(sb, h1)
                for b in range(0, mid):
                    b2.emit_gcn_block(sb, b)
                b1.emit_gcnw_half(sb, h2)
                for b in range(mid, st2["NBLK"]):
                    b2.emit_gcn_block(sb, b)
                ws1 = b1.emit_pooling(sb, pool_loc1, 0, plws[0])
            ag_pool1 = nc.gpsimd.collective_compute(
                "AllGather", mybir.AluOpType.bypass,
                replica_groups=[list(range(NC))],
                ins=[pool_loc1[:]], outs=[pool_glob1[:]])
            add_dep_helper(ag_pool1.ins, plws[0].ins, reason="AGp1 after init")
            for w in ws1:
                add_dep_helper(ag_pool1.ins, w.ins, reason="AGp1 after stages")

            # P5: GCN-W(b2) + pooling(b2); AG_pool(b2); tail
            with tc.tile_pool(name="p5", bufs=2) as sb:
                b2.alloc_zfin(sb)
                for blks in _halves(st2["NBLK"]):
                    b2.emit_gcnw_half(sb, blks)
                ws2 = b2.emit_pooling(sb, pool_loc2, 0, plws[1])
            ag_pool2 = nc.gpsimd.collective_compute(
                "AllGather", mybir.AluOpType.bypass,
                replica_groups=[list(range(NC))],
                ins=[pool_loc2[:]], outs=[pool_glob2[:]])
            add_dep_helper(ag_pool2.ins, plws[1].ins, reason="AGp2 after init")
            for w in ws2:
                add_dep_helper(ag_pool2.ins, w.ins, reason="AGp2 after stages")
            with tc.tile_pool(name="tail", bufs=2) as sb:
                pools["sb"] = sb
                _build_tail(nc, pools, (pool_glob1, pool_glob2),
                            (ag_pool1, ag_pool2))

    nc.compile()
    return nc


# ---------------------------------------------------------------- entry point
def kernel(**inputs) -> np.ndarray:
    st1, pc1, sh1 = _prep_branch(inputs["x1"], inputs["edge_index1"], inputs["batch1"],
                                 inputs["p1_gatW"], inputs["p1_att_src"],
                                 inputs["p1_att_dst"])
    st2, pc2, sh2 = _prep_branch(inputs["x2"], inputs["edge_index2"], inputs["batch2"],
                                 inputs["p2_gatW"], inputs["p2_att_src"],
                                 inputs["p2_att_dst"])
    gatWk1, gcnWk1 = _pack_branch_weights(inputs["p1_gatW"], inputs["p1_gatb"],
                                          inputs["p1_gcnW"], inputs["p1_gcnb"])
    gatWk2, gcnWk2 = _pack_branch_weights(inputs["p2_gatW"], inputs["p2_gatb"],
                                          inputs["p2_gcnW"], inputs["p2_gcnb"])
    tail = _pack_tail(inputs)

    key = (st1["MAXG"], st1["NSUBS"], st1["NGS"], st2["MAXG"], st2["NSUBS"],
           st2["NGS"])
    if key not in _PROG_CACHE:
        _PROG_CACHE[key] = _build_program(st1, st2)
    nc = _PROG_CACHE[key]

    in_maps = []
    for c in range(NC):
        m = {"identity": tail["identity"], "target": tail["target"],
             "fcxtWk": tail["fcxtWk"], "fc1Wk": tail["fc1Wk"],
             "fc2Wk": tail["fc2Wk"], "outWk": tail["outWk"],
             "p1_fcg1Wk": tail["p1_fcg1Wk"], "p1_fcg2Wk": tail["p1_fcg2Wk"],
             "p2_fcg1Wk": tail["p2_fcg1Wk"], "p2_fcg2Wk": tail["p2_fcg2Wk"]}
        for pfx, pc, sh, gatWk, gcnWk in (("b1_", pc1, sh1, gatWk1, gcnWk1),
                                          ("b2_", pc2, sh2, gatWk2, gcnWk2)):
            p = pc[c]
            m[pfx + "xslot"] = sh["xslot"]
            m[pfx + "xslotT"] = sh["xslotT"]
            m[pfx + "Wsd"] = sh["Wsd"]
            m[pfx + "gatWk"] = gatWk
            m[pfx + "gcnWk"] = gcnWk
            for k in ("isrc", "mask", "cols", "poison", "rcnt", "xownT"):
                m[pfx + k] = p[k]
        in_maps.append(m)

    res = run_bass_kernel_spmd(nc, in_maps, list(range(NC)))
    global LAST_RES
    LAST_RES = res
    return np.asarray(res.results[0]["out"], dtype=np.float32)


LAST_RES = None
